# revision 19
# baseline (speedup 1.0000x reference)
"""Trainium2 Bass kernel for batched multi-head attention with additive bias.

Reference computation (per batch b):
    q = (x @ Wq) * d**-0.5, k = x @ Wk, v = x @ Wv      (heads split, d=64, nh=12)
    scores = q @ k^T + attn_bias                         ([nh, N, N], bias broadcast)
    attn   = softmax(scores, axis=-1)                    (returned as output #2)
    out    = (attn @ v).merge_heads() @ Wproj            (returned as output #1)

Sharding: data-parallel over batch B=8 across the 8 NeuronCores (one batch
element per core, weights + bias replicated). No collectives needed.

Per-core dataflow (all matmuls in float32r = PE native fast fp32 mode):
  - host passes x[b]^T so qT/kT (transposed) and v (natural) come straight
    out of the PE without any on-device transpose of x.
  - scores are built per (head-pair, q-tile) in PSUM: K=64 QK^T matmuls for
    two heads packed into disjoint PE row groups, then the bias is
    accumulated into PSUM with an identity-matmul (bias in bf16 - its
    magnitude is 0.02 so bf16 rounding is ~1e-5 absolute).
  - softmax: no max-subtraction needed (scores are O(1), exp can't overflow
    in fp32). ScalarE Exp reads PSUM and accumulates the row sum for free
    (accum_out). VectorE computes reciprocals and multiplies rows by 1/sum,
    writing bf16 staging tiles (GpSimd only generates the cast-DMA
    descriptors - its compute path measured ~20x slower than DVE).
  - attn tiles are DMA'd out in 1MB chunks (SWDGE bf16->f32 cast) and also
    transposed on the PE via REGULAR matmuls against the identity (exact
    for bf16, and unlike transpose-mode they keep the HAM clock gate at
    2.4 GHz) into [k, q] layout feeding the attn @ v matmuls.
  - final projection contracts ctx^T against Wproj giving the output in
    natural [n, c] layout for a single contiguous DMA.
"""

import os

os.environ.setdefault("MYCRO_LOCAL_CACHE", "1")

import numpy as np
import ml_dtypes

B, N, C = 8, 1024, 768
NH, D = 12, 64
P = 128
F32R = None  # filled after mybir import

_CACHE = {}
LAST_EXEC_TIME_NS = None
LAST_RESULTS = None


def _build():
    from contextlib import ExitStack

    import concourse.bass as bass
    import concourse.tile as tile
    from concourse import bacc, mybir

    f32 = mybir.dt.float32
    f32r = mybir.dt.float32r
    bf16 = mybir.dt.bfloat16
    Exp = mybir.ActivationFunctionType.Exp
    mult = mybir.AluOpType.mult

    nc = bacc.Bacc(
        "TRN2", target_bir_lowering=False, debug=False, num_devices=8
    )

    xT_d = nc.dram_tensor("xT", [C, N], f32r, kind="ExternalInput")
    Wq_d = nc.dram_tensor("Wq", [C, C], f32r, kind="ExternalInput")
    Wk_d = nc.dram_tensor("Wk", [C, C], f32r, kind="ExternalInput")
    Wv_d = nc.dram_tensor("Wv", [C, C], f32r, kind="ExternalInput")
    Wp_d = nc.dram_tensor("Wproj", [C, C], f32r, kind="ExternalInput")
    bias_d = nc.dram_tensor("biasb", [N, N], bf16, kind="ExternalInput")
    out_d = nc.dram_tensor("out", [N, C], f32, kind="ExternalOutput")
    attn_d = nc.dram_tensor("attn", [NH, N, N], f32, kind="ExternalOutput")

    identb_d = nc.inline_tensor(
        np.eye(P, dtype=ml_dtypes.bfloat16), name="identb"
    )

    CT = C // P  # 6 column tiles of 128
    NT = N // P  # 8 seq tiles of 128
    NPAIR = NH // 2  # 6 head pairs; pair t occupies c' columns [t*128, t*128+128)

    with tile.TileContext(nc) as tc:
        with ExitStack() as ctx:
            pers = ctx.enter_context(tc.tile_pool(name="pers", bufs=1))
            qTs = pers.tile([P, CT, N], f32r)  # q^T, scaled by d**-0.5
            kTs = pers.tile([P, CT, N], f32r)  # k^T
            vs = pers.tile([P, NT, C], bf16)  # v natural [n, c'] (bf16 for PV)
            biass = pers.tile([P, NT, N], bf16)  # bias [q, k], q partition-tiled
            ctxT = pers.tile([P, CT, N], f32r)  # (attn@v)^T accumulated per pair
            sums = pers.tile([P, NT, NH], f32)
            recips = pers.tile([P, NT, NH], f32)
            identb = pers.tile([P, P], bf16)
            wpsb = pers.tile([P, CT, C], f32r)
            nc.sync.dma_start(
                out=wpsb[:], in_=Wp_d.ap().rearrange("(o p) f -> p o f", p=P)
            )

            # ---------------- Phase 1: QKV projections ----------------
            with ExitStack() as ph1:
                p1x = ph1.enter_context(tc.tile_pool(name="p1x", bufs=1))
                p1w = ph1.enter_context(tc.tile_pool(name="p1w", bufs=2))
                p1ps = ph1.enter_context(
                    tc.tile_pool(name="p1ps", bufs=2, space="PSUM")
                )

                xTs = p1x.tile([P, CT, N], f32r)
                xT_ap = xT_d.ap().rearrange("(o p) n -> p o n", p=P)
                for k in range(CT):
                    nc.sync.dma_start(out=xTs[:, k, :], in_=xT_ap[:, k, :])
                wq_first = True

                for w_d, mode in ((Wq_d, "q"), (Wk_d, "k"), (Wv_d, "v")):
                    wsb = p1w.tile([P, CT, C], f32r, tag="w")
                    w_ap = w_d.ap().rearrange("(o p) f -> p o f", p=P)
                    for k in range(CT):
                        nc.sync.dma_start(out=wsb[:, k, :], in_=w_ap[:, k, :])
                    if wq_first:
                        # queued after the tensors the first matmuls need
                        wq_first = False
                        nc.sync.dma_start(out=identb[:], in_=identb_d.ap())
                        nc.sync.dma_start(
                            out=biass[:],
                            in_=bias_d.ap().rearrange("(o p) k -> p o k", p=P),
                        )
                    if mode in ("q", "k"):
                        dst = qTs if mode == "q" else kTs
                        # dst[c', n] = sum_c W[c, c'] xT[c, n]
                        for m in range(CT):
                            for nh2 in range(2):
                                ps = p1ps.tile([P, C], f32, tag="p")
                                for k in range(CT):
                                    nc.tensor.matmul(
                                        ps[:, :512],
                                        lhsT=wsb[
                                            :, k, m * P : (m + 1) * P
                                        ],
                                        rhs=xTs[
                                            :, k, nh2 * 512 : nh2 * 512 + 512
                                        ],
                                        start=(k == 0),
                                        stop=(k == CT - 1),
                                    )
                                dslice = dst[:, m, nh2 * 512 : nh2 * 512 + 512]
                                if mode == "q":
                                    nc.vector.tensor_scalar_mul(
                                        dslice, ps[:, :512], float(D**-0.5)
                                    )
                                else:
                                    nc.any.tensor_copy(
                                        out=dslice, in_=ps[:, :512]
                                    )
                    else:
                        # v[n, c'] = sum_c xT[c, n] Wv[c, c']
                        for m in range(NT):
                            ps = p1ps.tile([P, C], f32, tag="p")
                            for off, nn in ((0, 512), (512, 256)):
                                for k in range(CT):
                                    nc.tensor.matmul(
                                        ps[:, off : off + nn],
                                        lhsT=xTs[
                                            :, k, m * P : (m + 1) * P
                                        ],
                                        rhs=wsb[:, k, off : off + nn],
                                        start=(k == 0),
                                        stop=(k == CT - 1),
                                    )
                            nc.any.tensor_copy(out=vs[:, m, :], in_=ps[:])

            # ---------------- Phase 2: attention per head pair ----------------
            with ExitStack() as ph2:
                s_ps = ph2.enter_context(
                    tc.tile_pool(name="s_ps", bufs=2, space="PSUM")
                )
                tp_ps = ph2.enter_context(
                    tc.tile_pool(name="tp_ps", bufs=2, space="PSUM")
                )
                pv_ps = ph2.enter_context(
                    tc.tile_pool(name="pv_ps", bufs=2, space="PSUM")
                )
                tmpb_pool = ph2.enter_context(tc.tile_pool(name="tmpb", bufs=2))
                e_pool = ph2.enter_context(tc.tile_pool(name="e", bufs=6))
                et_pool = ph2.enter_context(tc.tile_pool(name="et", bufs=2))
                as_pool = ph2.enter_context(tc.tile_pool(name="astg", bufs=3))

                attn_ap = attn_d.ap().rearrange("h (o p) k -> h p o k", p=P)

                for t in range(NPAIR):
                    hA, hB = 2 * t, 2 * t + 1
                    tmpB = tmpb_pool.tile([64, N], f32r, tag="tmpb")
                    for blk in range(4):
                        stA = as_pool.tile([P, 2, N], bf16, tag="astg")
                        stB = as_pool.tile([P, 2, N], bf16, tag="astg")
                        eT = et_pool.tile([P, NT, 512], bf16, tag="et")
                        for i in range(2):
                            qt = 2 * blk + i
                            psA = s_ps.tile([P, N], f32, tag="s")
                            psB = s_ps.tile([P, N], f32, tag="s")
                            for kh in range(2):
                                sl = slice(kh * 512, kh * 512 + 512)
                                # two heads packed into PE row groups 0-63 / 64-127
                                nc.tensor.matmul(
                                    psA[:, sl],
                                    lhsT=qTs[
                                        0:64, t, qt * P : (qt + 1) * P
                                    ],
                                    rhs=kTs[0:64, t, sl],
                                    start=True,
                                    stop=False,
                                    tile_position=(0, 0),
                                )
                                nc.tensor.matmul(
                                    psB[:, sl],
                                    lhsT=qTs[
                                        64:128, t, qt * P : (qt + 1) * P
                                    ],
                                    rhs=kTs[64:128, t, sl],
                                    start=True,
                                    stop=False,
                                    tile_position=(64, 0),
                                )
                                # additive bias via identity matmul accumulate
                                nc.tensor.matmul(
                                    psA[:, sl],
                                    lhsT=identb[:],
                                    rhs=biass[:, qt, sl],
                                    start=False,
                                    stop=True,
                                )
                                nc.tensor.matmul(
                                    psB[:, sl],
                                    lhsT=identb[:],
                                    rhs=biass[:, qt, sl],
                                    start=False,
                                    stop=True,
                                )
                            eA = e_pool.tile([P, N], f32, tag="e")
                            eB = e_pool.tile([P, N], f32, tag="e")
                            nc.scalar.activation(
                                eA[:],
                                psA[:],
                                Exp,
                                accum_out=sums[:, qt, hA : hA + 1],
                            )
                            nc.scalar.activation(
                                eB[:],
                                psB[:],
                                Exp,
                                accum_out=sums[:, qt, hB : hB + 1],
                            )
                            nc.vector.reciprocal(
                                recips[:, qt, hA : hA + 2],
                                sums[:, qt, hA : hA + 2],
                            )
                            # normalize on DVE (fp32 SBUF tensor_scalar runs
                            # in 2x mode; GpSimd measured 14.7us/tile here -
                            # ~20x slower - so it gets no work at all)
                            nc.vector.tensor_scalar(
                                stA[:, i, :],
                                eA[:],
                                recips[:, qt, hA : hA + 1],
                                None,
                                mult,
                            )
                            nc.vector.tensor_scalar(
                                stB[:, i, :],
                                eB[:],
                                recips[:, qt, hB : hB + 1],
                                None,
                                mult,
                            )
                        # stream normalized attention to DRAM (1MB per DMA,
                        # bf16->f32 cast done by the SWDGE path)
                        nc.gpsimd.dma_start(
                            out=attn_ap[hA, :, 2 * blk : 2 * blk + 2, :],
                            in_=stA[:],
                        )
                        nc.gpsimd.dma_start(
                            out=attn_ap[hB, :, 2 * blk : 2 * blk + 2, :],
                            in_=stB[:],
                        )
                        # transpose the 2 q-tiles x 2 heads into [k, q]
                        # layout via REGULAR matmuls against the identity
                        # (out = st_tile.T @ I, exact for bf16): unlike
                        # transpose-mode these count as matmul activity for
                        # the HAM clock gate, keeping the PE at 2.4 GHz.
                        # (A DMA-xbar version passed CoreSim but produced
                        # garbage on hardware - known sim/HW divergence.)
                        for kt in range(NT):
                            tp = tp_ps.tile([P, 512], f32, tag="tp")
                            for j, st in enumerate((stA, stA, stB, stB)):
                                i = j % 2
                                nc.tensor.matmul(
                                    tp[:, j * P : (j + 1) * P],
                                    lhsT=st[:, i, kt * P : (kt + 1) * P],
                                    rhs=identb[:],
                                    start=True,
                                    stop=True,
                                    skip_group_check=True,
                                )
                            nc.vector.tensor_copy(
                                out=eT[:, kt, :], in_=tp[:]
                            )
                        # attn @ v for this 256-wide q block: head A into
                        # cols 0:256, head B into cols 256:512 of one PSUM
                        # bank (partitions 0-63); accumulate over k-tiles.
                        # One start=True clears the bank's has_written bits;
                        # every later matmul overwrites untouched elements
                        # and accumulates written ones, so head B needs no
                        # start flag of its own.
                        pv = pv_ps.tile([64, 512], f32, tag="pv")
                        for kt in range(NT):
                            nc.tensor.matmul(
                                pv[:, 0:256],
                                lhsT=vs[:, kt, t * P : t * P + 64],
                                rhs=eT[:, kt, 0:256],
                                start=(kt == 0),
                                stop=False,
                                skip_group_check=True,
                            )
                            nc.tensor.matmul(
                                pv[:, 256:512],
                                lhsT=vs[:, kt, t * P + 64 : t * P + 128],
                                rhs=eT[:, kt, 256:512],
                                start=False,
                                stop=(kt == NT - 1),
                                skip_group_check=True,
                            )
                        bsl = slice(blk * 256, blk * 256 + 256)
                        nc.any.tensor_copy(
                            out=ctxT[0:64, t, bsl], in_=pv[:, 0:256]
                        )
                        nc.any.tensor_copy(
                            out=tmpB[:, bsl], in_=pv[:, 256:512]
                        )
                    # head B context lives at partitions 0-63; DMA shifts it
                    # to partitions 64-127 of ctxT (engines can't cross
                    # partitions, DMA can).
                    nc.sync.dma_start(out=ctxT[64:128, t, :], in_=tmpB[:])

            # ---------------- Phase 3: output projection ----------------
            with ExitStack() as ph3:
                p3 = ph3.enter_context(tc.tile_pool(name="p3", bufs=1))
                p3ps = ph3.enter_context(
                    tc.tile_pool(name="p3ps", bufs=2, space="PSUM")
                )
                outst = p3.tile([P, NT, C], f32)
                for m in range(NT):
                    ps = p3ps.tile([P, C], f32, tag="pp")
                    for off, nn in ((0, 512), (512, 256)):
                        for k in range(CT):
                            nc.tensor.matmul(
                                ps[:, off : off + nn],
                                lhsT=ctxT[:, k, m * P : (m + 1) * P],
                                rhs=wpsb[:, k, off : off + nn],
                                start=(k == 0),
                                stop=(k == CT - 1),
                            )
                    nc.any.tensor_copy(out=outst[:, m, :], in_=ps[:])
                nc.sync.dma_start(
                    out=out_d.ap().rearrange("(o p) c -> p o c", p=P),
                    in_=outst[:],
                )

    nc.compile()
    return nc


def _install_axon_ntff_hook():
    """Provide antenv.axon_hooks (absent in this image) so that
    run_bass_kernel_spmd(trace=True) can capture NTFF profiles through
    the axon sidechannel. Mirrors trn_agent_boot.trn_boot."""
    import sys
    import types
    import ctypes
    import contextlib

    if "antenv.axon_hooks" in sys.modules:
        return
    so_path = "/opt/axon/libaxon_pjrt.so"
    lib = ctypes.CDLL(so_path)
    if not hasattr(lib, "axon_start_nrt_profile"):
        return
    lib.axon_start_nrt_profile.argtypes = [
        ctypes.POINTER(ctypes.c_int64),
        ctypes.c_size_t,
    ]
    lib.axon_start_nrt_profile.restype = ctypes.c_int64
    lib.axon_stop_nrt_profile.argtypes = [ctypes.c_char_p]
    lib.axon_stop_nrt_profile.restype = ctypes.c_int64

    @contextlib.contextmanager
    def _hook(output_dir, device_ids):
        import jax

        jax.devices()
        if device_ids:
            ids = (ctypes.c_int64 * len(device_ids))(*device_ids)
            rc = lib.axon_start_nrt_profile(ids, len(device_ids))
        else:
            rc = lib.axon_start_nrt_profile(None, 0)
        if rc != 0:
            raise RuntimeError(f"axon_start_nrt_profile rc={rc}")
        try:
            yield
        finally:
            n = lib.axon_stop_nrt_profile(str(output_dir).encode())
            print(f"[kernel] ntff profile: {n} file(s) -> {output_dir}")

    mod = types.ModuleType("antenv.axon_hooks")
    mod.get_axon_ntff_profile_hook = lambda: _hook
    mod.set_axon_ntff_profile_hook = lambda h: None
    sys.modules["antenv.axon_hooks"] = mod


def kernel(x, Wq, Wk, Wv, Wproj, attn_bias):
    global LAST_EXEC_TIME_NS, LAST_RESULTS
    from concourse.bass_utils import run_bass_kernel_spmd

    if "nc" not in _CACHE:
        _CACHE["nc"] = _build()
    nc = _CACHE["nc"]

    x = np.ascontiguousarray(np.asarray(x, dtype=np.float32))
    bias_bf = np.ascontiguousarray(
        np.asarray(attn_bias, dtype=np.float32).astype(ml_dtypes.bfloat16)
    )
    Wq = np.ascontiguousarray(np.asarray(Wq, dtype=np.float32))
    Wk = np.ascontiguousarray(np.asarray(Wk, dtype=np.float32))
    Wv = np.ascontiguousarray(np.asarray(Wv, dtype=np.float32))
    Wproj = np.ascontiguousarray(np.asarray(Wproj, dtype=np.float32))

    in_maps = [
        {
            "xT": np.ascontiguousarray(x[b].T),
            "Wq": Wq,
            "Wk": Wk,
            "Wv": Wv,
            "Wproj": Wproj,
            "biasb": bias_bf,
        }
        for b in range(B)
    ]

    trace = os.environ.get("KERNEL_PROFILE", "0") == "1"
    res = None
    if trace:
        try:
            _install_axon_ntff_hook()
            tdir = os.environ.get("KERNEL_TRACE_DIR") or None
            res = run_bass_kernel_spmd(
                nc, in_maps, core_ids=list(range(B)), trace=True, tmpdir=tdir
            )
            LAST_EXEC_TIME_NS = res.exec_time_ns
        except Exception as exc:  # trace path can fail; retry without
            import traceback

            traceback.print_exc()
            print(f"[kernel] trace run failed ({exc!r}); rerunning untraced")
            res = None
    if res is None:
        res = run_bass_kernel_spmd(nc, in_maps, core_ids=list(range(B)))
    LAST_RESULTS = res

    out = np.stack([np.asarray(res.results[b]["out"]) for b in range(B)])
    attn = np.stack([np.asarray(res.results[b]["attn"]) for b in range(B)])
    return out, attn


# revision 20
# speedup vs baseline: 1.1893x; 1.1893x over previous
"""Trainium2 Bass kernel for batched multi-head attention with additive bias.

Reference computation (per batch b):
    q = (x @ Wq) * d**-0.5, k = x @ Wk, v = x @ Wv      (heads split, d=64, nh=12)
    scores = q @ k^T + attn_bias                         ([nh, N, N], bias broadcast)
    attn   = softmax(scores, axis=-1)                    (returned as output #2)
    out    = (attn @ v).merge_heads() @ Wproj            (returned as output #1)

Sharding: data-parallel over batch B=8 across the 8 NeuronCores (one batch
element per core, weights + bias replicated). No collectives needed.

Per-core dataflow (all matmuls in float32r = PE native fast fp32 mode):
  - host passes x[b]^T so qT/kT (transposed) and v (natural) come straight
    out of the PE without any on-device transpose of x.
  - scores are built per (head-pair, q-tile) in PSUM: K=64 QK^T matmuls for
    two heads packed into disjoint PE row groups, then the bias is
    accumulated into PSUM with an identity-matmul (bias in bf16 - its
    magnitude is 0.02 so bf16 rounding is ~1e-5 absolute).
  - softmax: no max-subtraction needed (scores are O(1), exp can't overflow
    in fp32). ScalarE Exp reads PSUM and accumulates the row sum for free
    (accum_out). VectorE computes reciprocals and multiplies rows by 1/sum,
    writing bf16 staging tiles (GpSimd only generates the cast-DMA
    descriptors - its compute path measured ~20x slower than DVE).
  - attn tiles are DMA'd out in 1MB chunks (SWDGE bf16->f32 cast) and also
    transposed on the PE via REGULAR matmuls against the identity (exact
    for bf16, and unlike transpose-mode they keep the HAM clock gate at
    2.4 GHz) into [k, q] layout feeding the attn @ v matmuls.
  - final projection contracts ctx^T against Wproj giving the output in
    natural [n, c] layout for a single contiguous DMA.
"""

import os

os.environ.setdefault("MYCRO_LOCAL_CACHE", "1")

import numpy as np
import ml_dtypes

B, N, C = 8, 1024, 768
NH, D = 12, 64
P = 128
F32R = None  # filled after mybir import

_CACHE = {}
LAST_EXEC_TIME_NS = None
LAST_RESULTS = None


def _build():
    from contextlib import ExitStack

    import concourse.bass as bass
    import concourse.tile as tile
    from concourse import bacc, mybir

    f32 = mybir.dt.float32
    f32r = mybir.dt.float32r
    bf16 = mybir.dt.bfloat16
    Exp = mybir.ActivationFunctionType.Exp
    mult = mybir.AluOpType.mult

    nc = bacc.Bacc(
        "TRN2", target_bir_lowering=False, debug=False, num_devices=8
    )

    xT_d = nc.dram_tensor("xT", [C, N], f32r, kind="ExternalInput")
    Wq_d = nc.dram_tensor("Wq", [C, C], f32r, kind="ExternalInput")
    Wk_d = nc.dram_tensor("Wk", [C, C], f32r, kind="ExternalInput")
    Wv_d = nc.dram_tensor("Wv", [C, C], f32r, kind="ExternalInput")
    Wp_d = nc.dram_tensor("Wproj", [C, C], f32r, kind="ExternalInput")
    bias_d = nc.dram_tensor("biasb", [N, N], bf16, kind="ExternalInput")
    out_d = nc.dram_tensor("out", [N, C], f32, kind="ExternalOutput")
    attn_d = nc.dram_tensor("attn", [NH, N, N], f32, kind="ExternalOutput")

    identb_d = nc.inline_tensor(
        np.eye(P, dtype=ml_dtypes.bfloat16), name="identb"
    )

    CT = C // P  # 6 column tiles of 128
    NT = N // P  # 8 seq tiles of 128
    NPAIR = NH // 2  # 6 head pairs; pair t occupies c' columns [t*128, t*128+128)

    with tile.TileContext(nc) as tc:
        with ExitStack() as ctx:
            pers = ctx.enter_context(tc.tile_pool(name="pers", bufs=1))
            qTs = pers.tile([P, CT, N], f32r)  # q^T, scaled by d**-0.5
            kTs = pers.tile([P, CT, N], f32r)  # k^T
            vs = pers.tile([P, NT, C], bf16)  # v natural [n, c'] (bf16 for PV)
            biass = pers.tile([P, NT, N], bf16)  # bias [q, k], q partition-tiled
            ctxT = pers.tile([P, CT, N], f32r)  # (attn@v)^T accumulated per pair
            sums = pers.tile([P, NT, NH], f32)
            recips = pers.tile([P, NT, NH], f32)
            identb = pers.tile([P, P], bf16)
            wpsb = pers.tile([P, CT, C], f32r)
            nc.sync.dma_start(
                out=wpsb[:], in_=Wp_d.ap().rearrange("(o p) f -> p o f", p=P)
            )

            # ---------------- Phase 1: QKV projections ----------------
            with ExitStack() as ph1:
                p1x = ph1.enter_context(tc.tile_pool(name="p1x", bufs=1))
                p1w = ph1.enter_context(tc.tile_pool(name="p1w", bufs=2))
                p1ps = ph1.enter_context(
                    tc.tile_pool(name="p1ps", bufs=2, space="PSUM")
                )

                xTs = p1x.tile([P, CT, N], f32r)
                xT_ap = xT_d.ap().rearrange("(o p) n -> p o n", p=P)
                for k in range(CT):
                    nc.sync.dma_start(out=xTs[:, k, :], in_=xT_ap[:, k, :])
                wq_first = True

                for w_d, mode in ((Wq_d, "q"), (Wk_d, "k"), (Wv_d, "v")):
                    wsb = p1w.tile([P, CT, C], f32r, tag="w")
                    w_ap = w_d.ap().rearrange("(o p) f -> p o f", p=P)
                    for k in range(CT):
                        nc.sync.dma_start(out=wsb[:, k, :], in_=w_ap[:, k, :])
                    if wq_first:
                        # queued after the tensors the first matmuls need
                        wq_first = False
                        nc.sync.dma_start(out=identb[:], in_=identb_d.ap())
                        nc.sync.dma_start(
                            out=biass[:],
                            in_=bias_d.ap().rearrange("(o p) k -> p o k", p=P),
                        )
                    if mode in ("q", "k"):
                        dst = qTs if mode == "q" else kTs
                        # dst[c', n] = sum_c W[c, c'] xT[c, n]
                        for m in range(CT):
                            for nh2 in range(2):
                                ps = p1ps.tile([P, C], f32, tag="p")
                                for k in range(CT):
                                    nc.tensor.matmul(
                                        ps[:, :512],
                                        lhsT=wsb[
                                            :, k, m * P : (m + 1) * P
                                        ],
                                        rhs=xTs[
                                            :, k, nh2 * 512 : nh2 * 512 + 512
                                        ],
                                        start=(k == 0),
                                        stop=(k == CT - 1),
                                    )
                                dslice = dst[:, m, nh2 * 512 : nh2 * 512 + 512]
                                if mode == "q":
                                    nc.vector.tensor_scalar_mul(
                                        dslice, ps[:, :512], float(D**-0.5)
                                    )
                                else:
                                    nc.any.tensor_copy(
                                        out=dslice, in_=ps[:, :512]
                                    )
                    else:
                        # v[n, c'] = sum_c xT[c, n] Wv[c, c']
                        for m in range(NT):
                            ps = p1ps.tile([P, C], f32, tag="p")
                            for off, nn in ((0, 512), (512, 256)):
                                for k in range(CT):
                                    nc.tensor.matmul(
                                        ps[:, off : off + nn],
                                        lhsT=xTs[
                                            :, k, m * P : (m + 1) * P
                                        ],
                                        rhs=wsb[:, k, off : off + nn],
                                        start=(k == 0),
                                        stop=(k == CT - 1),
                                    )
                            nc.any.tensor_copy(out=vs[:, m, :], in_=ps[:])

            # ---------------- Phase 2: attention per head pair ----------------
            with ExitStack() as ph2:
                s_ps = ph2.enter_context(
                    tc.tile_pool(name="s_ps", bufs=2, space="PSUM")
                )
                tp_ps = ph2.enter_context(
                    tc.tile_pool(name="tp_ps", bufs=2, space="PSUM")
                )
                pv_ps = ph2.enter_context(
                    tc.tile_pool(name="pv_ps", bufs=2, space="PSUM")
                )
                tmpb_pool = ph2.enter_context(tc.tile_pool(name="tmpb", bufs=2))
                e_pool = ph2.enter_context(tc.tile_pool(name="e", bufs=6))
                et_pool = ph2.enter_context(tc.tile_pool(name="et", bufs=1))
                as_pool = ph2.enter_context(tc.tile_pool(name="astg", bufs=3))

                attn_ap = attn_d.ap().rearrange("h (o p) k -> h p o k", p=P)

                for t in range(NPAIR):
                    hA, hB = 2 * t, 2 * t + 1
                    tmpB = tmpb_pool.tile([64, N], f32r, tag="tmpb")
                    for blk in range(4):
                        stA = as_pool.tile([P, 2, N], bf16, tag="astg")
                        stB = as_pool.tile([P, 2, N], bf16, tag="astg")
                        eT = et_pool.tile([P, NT, 512], bf16, tag="et")
                        for i in range(2):
                            qt = 2 * blk + i
                            psA = s_ps.tile([P, N], f32, tag="s")
                            psB = s_ps.tile([P, N], f32, tag="s")
                            for kh in range(2):
                                sl = slice(kh * 512, kh * 512 + 512)
                                # two heads packed into PE row groups 0-63 / 64-127
                                nc.tensor.matmul(
                                    psA[:, sl],
                                    lhsT=qTs[
                                        0:64, t, qt * P : (qt + 1) * P
                                    ],
                                    rhs=kTs[0:64, t, sl],
                                    start=True,
                                    stop=False,
                                    tile_position=(0, 0),
                                )
                                nc.tensor.matmul(
                                    psB[:, sl],
                                    lhsT=qTs[
                                        64:128, t, qt * P : (qt + 1) * P
                                    ],
                                    rhs=kTs[64:128, t, sl],
                                    start=True,
                                    stop=False,
                                    tile_position=(64, 0),
                                )
                                # additive bias via identity matmul accumulate
                                nc.tensor.matmul(
                                    psA[:, sl],
                                    lhsT=identb[:],
                                    rhs=biass[:, qt, sl],
                                    start=False,
                                    stop=True,
                                )
                                nc.tensor.matmul(
                                    psB[:, sl],
                                    lhsT=identb[:],
                                    rhs=biass[:, qt, sl],
                                    start=False,
                                    stop=True,
                                )
                            eA = e_pool.tile([P, N], f32, tag="e")
                            eB = e_pool.tile([P, N], f32, tag="e")
                            nc.scalar.activation(
                                eA[:],
                                psA[:],
                                Exp,
                                accum_out=sums[:, qt, hA : hA + 1],
                            )
                            nc.scalar.activation(
                                eB[:],
                                psB[:],
                                Exp,
                                accum_out=sums[:, qt, hB : hB + 1],
                            )
                            nc.vector.reciprocal(
                                recips[:, qt, hA : hA + 2],
                                sums[:, qt, hA : hA + 2],
                            )
                            # normalize on DVE (fp32 SBUF tensor_scalar runs
                            # in 2x mode; GpSimd measured 14.7us/tile here -
                            # ~20x slower - so it gets no work at all)
                            nc.vector.tensor_scalar(
                                stA[:, i, :],
                                eA[:],
                                recips[:, qt, hA : hA + 1],
                                None,
                                mult,
                            )
                            nc.vector.tensor_scalar(
                                stB[:, i, :],
                                eB[:],
                                recips[:, qt, hB : hB + 1],
                                None,
                                mult,
                            )
                        # stream normalized attention to DRAM (1MB per DMA,
                        # bf16->f32 cast done by the SWDGE path)
                        nc.gpsimd.dma_start(
                            out=attn_ap[hA, :, 2 * blk : 2 * blk + 2, :],
                            in_=stA[:],
                        )
                        nc.gpsimd.dma_start(
                            out=attn_ap[hB, :, 2 * blk : 2 * blk + 2, :],
                            in_=stB[:],
                        )
                        # transpose the 2 q-tiles x 2 heads into [k, q]
                        # layout via REGULAR matmuls against the identity
                        # (out = st_tile.T @ I, exact for bf16): unlike
                        # transpose-mode these count as matmul activity for
                        # the HAM clock gate, keeping the PE at 2.4 GHz.
                        # (A DMA-xbar version passed CoreSim but produced
                        # garbage on hardware - known sim/HW divergence.)
                        for kt in range(NT):
                            tp = tp_ps.tile([P, 512], f32, tag="tp")
                            for j, st in enumerate((stA, stA, stB, stB)):
                                i = j % 2
                                nc.tensor.matmul(
                                    tp[:, j * P : (j + 1) * P],
                                    lhsT=st[:, i, kt * P : (kt + 1) * P],
                                    rhs=identb[:],
                                    start=True,
                                    stop=True,
                                    skip_group_check=True,
                                )
                            if kt % 2 == 0:
                                nc.scalar.copy(out=eT[:, kt, :], in_=tp[:])
                            else:
                                nc.vector.tensor_copy(
                                    out=eT[:, kt, :], in_=tp[:]
                                )
                        # attn @ v for this 256-wide q block: head A into
                        # cols 0:256, head B into cols 256:512 of one PSUM
                        # bank (partitions 0-63); accumulate over k-tiles.
                        # One start=True clears the bank's has_written bits;
                        # every later matmul overwrites untouched elements
                        # and accumulates written ones, so head B needs no
                        # start flag of its own.
                        pv = pv_ps.tile([64, 512], f32, tag="pv")
                        for kt in range(NT):
                            nc.tensor.matmul(
                                pv[:, 0:256],
                                lhsT=vs[:, kt, t * P : t * P + 64],
                                rhs=eT[:, kt, 0:256],
                                start=(kt == 0),
                                stop=False,
                                skip_group_check=True,
                            )
                            nc.tensor.matmul(
                                pv[:, 256:512],
                                lhsT=vs[:, kt, t * P + 64 : t * P + 128],
                                rhs=eT[:, kt, 256:512],
                                start=False,
                                stop=(kt == NT - 1),
                                skip_group_check=True,
                            )
                        bsl = slice(blk * 256, blk * 256 + 256)
                        nc.any.tensor_copy(
                            out=ctxT[0:64, t, bsl], in_=pv[:, 0:256]
                        )
                        nc.any.tensor_copy(
                            out=tmpB[:, bsl], in_=pv[:, 256:512]
                        )
                    # head B context lives at partitions 0-63; DMA shifts it
                    # to partitions 64-127 of ctxT (engines can't cross
                    # partitions, DMA can).
                    nc.sync.dma_start(out=ctxT[64:128, t, :], in_=tmpB[:])

            # ---------------- Phase 3: output projection ----------------
            with ExitStack() as ph3:
                p3 = ph3.enter_context(tc.tile_pool(name="p3", bufs=1))
                p3ps = ph3.enter_context(
                    tc.tile_pool(name="p3ps", bufs=2, space="PSUM")
                )
                outst = p3.tile([P, NT, C], f32)
                for m in range(NT):
                    ps = p3ps.tile([P, C], f32, tag="pp")
                    for off, nn in ((0, 512), (512, 256)):
                        for k in range(CT):
                            nc.tensor.matmul(
                                ps[:, off : off + nn],
                                lhsT=ctxT[:, k, m * P : (m + 1) * P],
                                rhs=wpsb[:, k, off : off + nn],
                                start=(k == 0),
                                stop=(k == CT - 1),
                            )
                    nc.any.tensor_copy(out=outst[:, m, :], in_=ps[:])
                nc.sync.dma_start(
                    out=out_d.ap().rearrange("(o p) c -> p o c", p=P),
                    in_=outst[:],
                )

    nc.compile()
    return nc


def _install_axon_ntff_hook():
    """Provide antenv.axon_hooks (absent in this image) so that
    run_bass_kernel_spmd(trace=True) can capture NTFF profiles through
    the axon sidechannel. Mirrors trn_agent_boot.trn_boot."""
    import sys
    import types
    import ctypes
    import contextlib

    if "antenv.axon_hooks" in sys.modules:
        return
    so_path = "/opt/axon/libaxon_pjrt.so"
    lib = ctypes.CDLL(so_path)
    if not hasattr(lib, "axon_start_nrt_profile"):
        return
    lib.axon_start_nrt_profile.argtypes = [
        ctypes.POINTER(ctypes.c_int64),
        ctypes.c_size_t,
    ]
    lib.axon_start_nrt_profile.restype = ctypes.c_int64
    lib.axon_stop_nrt_profile.argtypes = [ctypes.c_char_p]
    lib.axon_stop_nrt_profile.restype = ctypes.c_int64

    @contextlib.contextmanager
    def _hook(output_dir, device_ids):
        import jax

        jax.devices()
        if device_ids:
            ids = (ctypes.c_int64 * len(device_ids))(*device_ids)
            rc = lib.axon_start_nrt_profile(ids, len(device_ids))
        else:
            rc = lib.axon_start_nrt_profile(None, 0)
        if rc != 0:
            raise RuntimeError(f"axon_start_nrt_profile rc={rc}")
        try:
            yield
        finally:
            n = lib.axon_stop_nrt_profile(str(output_dir).encode())
            print(f"[kernel] ntff profile: {n} file(s) -> {output_dir}")

    mod = types.ModuleType("antenv.axon_hooks")
    mod.get_axon_ntff_profile_hook = lambda: _hook
    mod.set_axon_ntff_profile_hook = lambda h: None
    sys.modules["antenv.axon_hooks"] = mod


def kernel(x, Wq, Wk, Wv, Wproj, attn_bias):
    global LAST_EXEC_TIME_NS, LAST_RESULTS
    from concourse.bass_utils import run_bass_kernel_spmd

    if "nc" not in _CACHE:
        _CACHE["nc"] = _build()
    nc = _CACHE["nc"]

    x = np.ascontiguousarray(np.asarray(x, dtype=np.float32))
    bias_bf = np.ascontiguousarray(
        np.asarray(attn_bias, dtype=np.float32).astype(ml_dtypes.bfloat16)
    )
    Wq = np.ascontiguousarray(np.asarray(Wq, dtype=np.float32))
    Wk = np.ascontiguousarray(np.asarray(Wk, dtype=np.float32))
    Wv = np.ascontiguousarray(np.asarray(Wv, dtype=np.float32))
    Wproj = np.ascontiguousarray(np.asarray(Wproj, dtype=np.float32))

    in_maps = [
        {
            "xT": np.ascontiguousarray(x[b].T),
            "Wq": Wq,
            "Wk": Wk,
            "Wv": Wv,
            "Wproj": Wproj,
            "biasb": bias_bf,
        }
        for b in range(B)
    ]

    trace = os.environ.get("KERNEL_PROFILE", "0") == "1"
    res = None
    if trace:
        try:
            _install_axon_ntff_hook()
            tdir = os.environ.get("KERNEL_TRACE_DIR") or None
            res = run_bass_kernel_spmd(
                nc, in_maps, core_ids=list(range(B)), trace=True, tmpdir=tdir
            )
            LAST_EXEC_TIME_NS = res.exec_time_ns
        except Exception as exc:  # trace path can fail; retry without
            import traceback

            traceback.print_exc()
            print(f"[kernel] trace run failed ({exc!r}); rerunning untraced")
            res = None
    if res is None:
        res = run_bass_kernel_spmd(nc, in_maps, core_ids=list(range(B)))
    LAST_RESULTS = res

    out = np.stack([np.asarray(res.results[b]["out"]) for b in range(B)])
    attn = np.stack([np.asarray(res.results[b]["attn"]) for b in range(B)])
    return out, attn


# revision 21
# speedup vs baseline: 1.1965x; 1.0060x over previous
"""Trainium2 Bass kernel for batched multi-head attention with additive bias.

Reference computation (per batch b):
    q = (x @ Wq) * d**-0.5, k = x @ Wk, v = x @ Wv      (heads split, d=64, nh=12)
    scores = q @ k^T + attn_bias                         ([nh, N, N], bias broadcast)
    attn   = softmax(scores, axis=-1)                    (returned as output #2)
    out    = (attn @ v).merge_heads() @ Wproj            (returned as output #1)

Sharding: data-parallel over batch B=8 across the 8 NeuronCores (one batch
element per core, weights + bias replicated). No collectives needed.

Per-core dataflow (all matmuls in float32r = PE native fast fp32 mode):
  - host passes x[b]^T so qT/kT (transposed) and v (natural) come straight
    out of the PE without any on-device transpose of x.
  - scores are built per (head-pair, q-tile) in PSUM: K=64 QK^T matmuls for
    two heads packed into disjoint PE row groups, then the bias is
    accumulated into PSUM with an identity-matmul (bias in bf16 - its
    magnitude is 0.02 so bf16 rounding is ~1e-5 absolute).
  - softmax: no max-subtraction needed (scores are O(1), exp can't overflow
    in fp32). ScalarE Exp reads PSUM and accumulates the row sum for free
    (accum_out). VectorE computes reciprocals and multiplies rows by 1/sum,
    writing bf16 staging tiles (GpSimd only generates the cast-DMA
    descriptors - its compute path measured ~20x slower than DVE).
  - attn tiles are DMA'd out in 1MB chunks (SWDGE bf16->f32 cast) and also
    transposed on the PE via REGULAR matmuls against the identity (exact
    for bf16, and unlike transpose-mode they keep the HAM clock gate at
    2.4 GHz) into [k, q] layout feeding the attn @ v matmuls.
  - final projection contracts ctx^T against Wproj giving the output in
    natural [n, c] layout for a single contiguous DMA.
"""

import os

os.environ.setdefault("MYCRO_LOCAL_CACHE", "1")

import numpy as np
import ml_dtypes

B, N, C = 8, 1024, 768
NH, D = 12, 64
P = 128
F32R = None  # filled after mybir import

_CACHE = {}
LAST_EXEC_TIME_NS = None
LAST_RESULTS = None


def _build():
    from contextlib import ExitStack

    import concourse.bass as bass
    import concourse.tile as tile
    from concourse import bacc, mybir

    f32 = mybir.dt.float32
    f32r = mybir.dt.float32r
    bf16 = mybir.dt.bfloat16
    Exp = mybir.ActivationFunctionType.Exp
    mult = mybir.AluOpType.mult

    nc = bacc.Bacc(
        "TRN2", target_bir_lowering=False, debug=False, num_devices=8
    )

    xT_d = nc.dram_tensor("xT", [C, N], f32r, kind="ExternalInput")
    Wq_d = nc.dram_tensor("Wq", [C, C], f32r, kind="ExternalInput")
    Wk_d = nc.dram_tensor("Wk", [C, C], f32r, kind="ExternalInput")
    Wv_d = nc.dram_tensor("Wv", [C, C], f32r, kind="ExternalInput")
    Wp_d = nc.dram_tensor("Wproj", [C, C], f32r, kind="ExternalInput")
    bias_d = nc.dram_tensor("biasb", [N, N], bf16, kind="ExternalInput")
    out_d = nc.dram_tensor("out", [N, C], f32, kind="ExternalOutput")
    attn_d = nc.dram_tensor("attn", [NH, N, N], f32, kind="ExternalOutput")

    identb_d = nc.inline_tensor(
        np.eye(P, dtype=ml_dtypes.bfloat16), name="identb"
    )

    CT = C // P  # 6 column tiles of 128
    NT = N // P  # 8 seq tiles of 128
    NPAIR = NH // 2  # 6 head pairs; pair t occupies c' columns [t*128, t*128+128)

    with tile.TileContext(nc) as tc:
        with ExitStack() as ctx:
            pers = ctx.enter_context(tc.tile_pool(name="pers", bufs=1))
            qTs = pers.tile([P, CT, N], f32r)  # q^T, scaled by d**-0.5
            kTs = pers.tile([P, CT, N], f32r)  # k^T
            vs = pers.tile([P, NT, C], bf16)  # v natural [n, c'] (bf16 for PV)
            biass = pers.tile([P, NT, N], bf16)  # bias [q, k], q partition-tiled
            ctxT = pers.tile([P, CT, N], f32r)  # (attn@v)^T accumulated per pair
            sums = pers.tile([P, NT, NH], f32)
            recips = pers.tile([P, NT, NH], f32)
            identb = pers.tile([P, P], bf16)
            wpsb = pers.tile([P, CT, C], f32r)
            nc.sync.dma_start(
                out=wpsb[:], in_=Wp_d.ap().rearrange("(o p) f -> p o f", p=P)
            )

            # ---------------- Phase 1: QKV projections ----------------
            with ExitStack() as ph1:
                p1x = ph1.enter_context(tc.tile_pool(name="p1x", bufs=1))
                p1w = ph1.enter_context(tc.tile_pool(name="p1w", bufs=2))
                p1ps = ph1.enter_context(
                    tc.tile_pool(name="p1ps", bufs=2, space="PSUM")
                )

                xTs = p1x.tile([P, CT, N], f32r)
                xT_ap = xT_d.ap().rearrange("(o p) n -> p o n", p=P)
                for k in range(CT):
                    nc.sync.dma_start(out=xTs[:, k, :], in_=xT_ap[:, k, :])
                wq_first = True

                for w_d, mode in ((Wq_d, "q"), (Wk_d, "k"), (Wv_d, "v")):
                    wsb = p1w.tile([P, CT, C], f32r, tag="w")
                    w_ap = w_d.ap().rearrange("(o p) f -> p o f", p=P)
                    for k in range(CT):
                        nc.sync.dma_start(out=wsb[:, k, :], in_=w_ap[:, k, :])
                    if wq_first:
                        # queued after the tensors the first matmuls need
                        wq_first = False
                        nc.sync.dma_start(out=identb[:], in_=identb_d.ap())
                        nc.sync.dma_start(
                            out=biass[:],
                            in_=bias_d.ap().rearrange("(o p) k -> p o k", p=P),
                        )
                    if mode in ("q", "k"):
                        dst = qTs if mode == "q" else kTs
                        # dst[c', n] = sum_c W[c, c'] xT[c, n]
                        for m in range(CT):
                            for nh2 in range(2):
                                ps = p1ps.tile([P, C], f32, tag="p")
                                for k in range(CT):
                                    nc.tensor.matmul(
                                        ps[:, :512],
                                        lhsT=wsb[
                                            :, k, m * P : (m + 1) * P
                                        ],
                                        rhs=xTs[
                                            :, k, nh2 * 512 : nh2 * 512 + 512
                                        ],
                                        start=(k == 0),
                                        stop=(k == CT - 1),
                                    )
                                dslice = dst[:, m, nh2 * 512 : nh2 * 512 + 512]
                                if mode == "q":
                                    nc.vector.tensor_scalar_mul(
                                        dslice, ps[:, :512], float(D**-0.5)
                                    )
                                else:
                                    nc.any.tensor_copy(
                                        out=dslice, in_=ps[:, :512]
                                    )
                    else:
                        # v[n, c'] = sum_c xT[c, n] Wv[c, c']
                        for m in range(NT):
                            ps = p1ps.tile([P, C], f32, tag="p")
                            for off, nn in ((0, 512), (512, 256)):
                                for k in range(CT):
                                    nc.tensor.matmul(
                                        ps[:, off : off + nn],
                                        lhsT=xTs[
                                            :, k, m * P : (m + 1) * P
                                        ],
                                        rhs=wsb[:, k, off : off + nn],
                                        start=(k == 0),
                                        stop=(k == CT - 1),
                                    )
                            nc.any.tensor_copy(out=vs[:, m, :], in_=ps[:])

            # ---------------- Phase 2: attention per head pair ----------------
            with ExitStack() as ph2:
                s_ps = ph2.enter_context(
                    tc.tile_pool(name="s_ps", bufs=2, space="PSUM")
                )
                tp_ps = ph2.enter_context(
                    tc.tile_pool(name="tp_ps", bufs=3, space="PSUM")
                )
                pv_ps = ph2.enter_context(
                    tc.tile_pool(name="pv_ps", bufs=1, space="PSUM")
                )
                tmpb_pool = ph2.enter_context(tc.tile_pool(name="tmpb", bufs=2))
                e_pool = ph2.enter_context(tc.tile_pool(name="e", bufs=6))
                et_pool = ph2.enter_context(tc.tile_pool(name="et", bufs=2))
                as_pool = ph2.enter_context(tc.tile_pool(name="astg", bufs=3))

                attn_ap = attn_d.ap().rearrange("h (o p) k -> h p o k", p=P)

                for t in range(NPAIR):
                    hA, hB = 2 * t, 2 * t + 1
                    tmpB = tmpb_pool.tile([64, N], f32r, tag="tmpb")
                    for blk in range(4):
                        stA = as_pool.tile([P, 2, N], bf16, tag="astg")
                        stB = as_pool.tile([P, 2, N], bf16, tag="astg")
                        eT = et_pool.tile([P, NT, 512], bf16, tag="et")
                        for i in range(2):
                            qt = 2 * blk + i
                            psA = s_ps.tile([P, N], f32, tag="s")
                            psB = s_ps.tile([P, N], f32, tag="s")
                            for kh in range(2):
                                sl = slice(kh * 512, kh * 512 + 512)
                                # two heads packed into PE row groups 0-63 / 64-127
                                nc.tensor.matmul(
                                    psA[:, sl],
                                    lhsT=qTs[
                                        0:64, t, qt * P : (qt + 1) * P
                                    ],
                                    rhs=kTs[0:64, t, sl],
                                    start=True,
                                    stop=False,
                                    tile_position=(0, 0),
                                )
                                nc.tensor.matmul(
                                    psB[:, sl],
                                    lhsT=qTs[
                                        64:128, t, qt * P : (qt + 1) * P
                                    ],
                                    rhs=kTs[64:128, t, sl],
                                    start=True,
                                    stop=False,
                                    tile_position=(64, 0),
                                )
                                # additive bias via identity matmul accumulate
                                nc.tensor.matmul(
                                    psA[:, sl],
                                    lhsT=identb[:],
                                    rhs=biass[:, qt, sl],
                                    start=False,
                                    stop=True,
                                )
                                nc.tensor.matmul(
                                    psB[:, sl],
                                    lhsT=identb[:],
                                    rhs=biass[:, qt, sl],
                                    start=False,
                                    stop=True,
                                )
                            eA = e_pool.tile([P, N], f32, tag="e")
                            eB = e_pool.tile([P, N], f32, tag="e")
                            nc.scalar.activation(
                                eA[:],
                                psA[:],
                                Exp,
                                accum_out=sums[:, qt, hA : hA + 1],
                            )
                            nc.scalar.activation(
                                eB[:],
                                psB[:],
                                Exp,
                                accum_out=sums[:, qt, hB : hB + 1],
                            )
                            nc.vector.reciprocal(
                                recips[:, qt, hA : hA + 2],
                                sums[:, qt, hA : hA + 2],
                            )
                            # normalize on DVE (fp32 SBUF tensor_scalar runs
                            # in 2x mode; GpSimd measured 14.7us/tile here -
                            # ~20x slower - so it gets no work at all)
                            nc.vector.tensor_scalar(
                                stA[:, i, :],
                                eA[:],
                                recips[:, qt, hA : hA + 1],
                                None,
                                mult,
                            )
                            nc.vector.tensor_scalar(
                                stB[:, i, :],
                                eB[:],
                                recips[:, qt, hB : hB + 1],
                                None,
                                mult,
                            )
                        # stream normalized attention to DRAM (1MB per DMA,
                        # bf16->f32 cast done by the SWDGE path)
                        nc.gpsimd.dma_start(
                            out=attn_ap[hA, :, 2 * blk : 2 * blk + 2, :],
                            in_=stA[:],
                        )
                        nc.gpsimd.dma_start(
                            out=attn_ap[hB, :, 2 * blk : 2 * blk + 2, :],
                            in_=stB[:],
                        )
                        # transpose the 2 q-tiles x 2 heads into [k, q]
                        # layout via REGULAR matmuls against the identity
                        # (out = st_tile.T @ I, exact for bf16): unlike
                        # transpose-mode these count as matmul activity for
                        # the HAM clock gate, keeping the PE at 2.4 GHz.
                        # (A DMA-xbar version passed CoreSim but produced
                        # garbage on hardware - known sim/HW divergence.)
                        for kt in range(NT):
                            tp = tp_ps.tile([P, 512], f32, tag="tp")
                            for j, st in enumerate((stA, stA, stB, stB)):
                                i = j % 2
                                nc.tensor.matmul(
                                    tp[:, j * P : (j + 1) * P],
                                    lhsT=st[:, i, kt * P : (kt + 1) * P],
                                    rhs=identb[:],
                                    start=True,
                                    stop=True,
                                    skip_group_check=True,
                                )
                            if kt % 2 == 0:
                                nc.scalar.copy(out=eT[:, kt, :], in_=tp[:])
                            else:
                                nc.vector.tensor_copy(
                                    out=eT[:, kt, :], in_=tp[:]
                                )
                        # attn @ v for this 256-wide q block: head A into
                        # cols 0:256, head B into cols 256:512 of one PSUM
                        # bank (partitions 0-63); accumulate over k-tiles.
                        # One start=True clears the bank's has_written bits;
                        # every later matmul overwrites untouched elements
                        # and accumulates written ones, so head B needs no
                        # start flag of its own.
                        pv = pv_ps.tile([64, 512], f32, tag="pv")
                        for kt in range(NT):
                            nc.tensor.matmul(
                                pv[:, 0:256],
                                lhsT=vs[:, kt, t * P : t * P + 64],
                                rhs=eT[:, kt, 0:256],
                                start=(kt == 0),
                                stop=False,
                                skip_group_check=True,
                            )
                            nc.tensor.matmul(
                                pv[:, 256:512],
                                lhsT=vs[:, kt, t * P + 64 : t * P + 128],
                                rhs=eT[:, kt, 256:512],
                                start=False,
                                stop=(kt == NT - 1),
                                skip_group_check=True,
                            )
                        bsl = slice(blk * 256, blk * 256 + 256)
                        nc.any.tensor_copy(
                            out=ctxT[0:64, t, bsl], in_=pv[:, 0:256]
                        )
                        nc.any.tensor_copy(
                            out=tmpB[:, bsl], in_=pv[:, 256:512]
                        )
                    # head B context lives at partitions 0-63; DMA shifts it
                    # to partitions 64-127 of ctxT (engines can't cross
                    # partitions, DMA can).
                    nc.sync.dma_start(out=ctxT[64:128, t, :], in_=tmpB[:])

            # ---------------- Phase 3: output projection ----------------
            with ExitStack() as ph3:
                p3 = ph3.enter_context(tc.tile_pool(name="p3", bufs=1))
                p3ps = ph3.enter_context(
                    tc.tile_pool(name="p3ps", bufs=2, space="PSUM")
                )
                outst = p3.tile([P, NT, C], f32)
                for m in range(NT):
                    ps = p3ps.tile([P, C], f32, tag="pp")
                    for off, nn in ((0, 512), (512, 256)):
                        for k in range(CT):
                            nc.tensor.matmul(
                                ps[:, off : off + nn],
                                lhsT=ctxT[:, k, m * P : (m + 1) * P],
                                rhs=wpsb[:, k, off : off + nn],
                                start=(k == 0),
                                stop=(k == CT - 1),
                            )
                    nc.any.tensor_copy(out=outst[:, m, :], in_=ps[:])
                nc.sync.dma_start(
                    out=out_d.ap().rearrange("(o p) c -> p o c", p=P),
                    in_=outst[:],
                )

    nc.compile()
    return nc


def _install_axon_ntff_hook():
    """Provide antenv.axon_hooks (absent in this image) so that
    run_bass_kernel_spmd(trace=True) can capture NTFF profiles through
    the axon sidechannel. Mirrors trn_agent_boot.trn_boot."""
    import sys
    import types
    import ctypes
    import contextlib

    if "antenv.axon_hooks" in sys.modules:
        return
    so_path = "/opt/axon/libaxon_pjrt.so"
    lib = ctypes.CDLL(so_path)
    if not hasattr(lib, "axon_start_nrt_profile"):
        return
    lib.axon_start_nrt_profile.argtypes = [
        ctypes.POINTER(ctypes.c_int64),
        ctypes.c_size_t,
    ]
    lib.axon_start_nrt_profile.restype = ctypes.c_int64
    lib.axon_stop_nrt_profile.argtypes = [ctypes.c_char_p]
    lib.axon_stop_nrt_profile.restype = ctypes.c_int64

    @contextlib.contextmanager
    def _hook(output_dir, device_ids):
        import jax

        jax.devices()
        if device_ids:
            ids = (ctypes.c_int64 * len(device_ids))(*device_ids)
            rc = lib.axon_start_nrt_profile(ids, len(device_ids))
        else:
            rc = lib.axon_start_nrt_profile(None, 0)
        if rc != 0:
            raise RuntimeError(f"axon_start_nrt_profile rc={rc}")
        try:
            yield
        finally:
            n = lib.axon_stop_nrt_profile(str(output_dir).encode())
            print(f"[kernel] ntff profile: {n} file(s) -> {output_dir}")

    mod = types.ModuleType("antenv.axon_hooks")
    mod.get_axon_ntff_profile_hook = lambda: _hook
    mod.set_axon_ntff_profile_hook = lambda h: None
    sys.modules["antenv.axon_hooks"] = mod


def kernel(x, Wq, Wk, Wv, Wproj, attn_bias):
    global LAST_EXEC_TIME_NS, LAST_RESULTS
    from concourse.bass_utils import run_bass_kernel_spmd

    if "nc" not in _CACHE:
        _CACHE["nc"] = _build()
    nc = _CACHE["nc"]

    x = np.ascontiguousarray(np.asarray(x, dtype=np.float32))
    bias_bf = np.ascontiguousarray(
        np.asarray(attn_bias, dtype=np.float32).astype(ml_dtypes.bfloat16)
    )
    Wq = np.ascontiguousarray(np.asarray(Wq, dtype=np.float32))
    Wk = np.ascontiguousarray(np.asarray(Wk, dtype=np.float32))
    Wv = np.ascontiguousarray(np.asarray(Wv, dtype=np.float32))
    Wproj = np.ascontiguousarray(np.asarray(Wproj, dtype=np.float32))

    in_maps = [
        {
            "xT": np.ascontiguousarray(x[b].T),
            "Wq": Wq,
            "Wk": Wk,
            "Wv": Wv,
            "Wproj": Wproj,
            "biasb": bias_bf,
        }
        for b in range(B)
    ]

    trace = os.environ.get("KERNEL_PROFILE", "0") == "1"
    res = None
    if trace:
        try:
            _install_axon_ntff_hook()
            tdir = os.environ.get("KERNEL_TRACE_DIR") or None
            res = run_bass_kernel_spmd(
                nc, in_maps, core_ids=list(range(B)), trace=True, tmpdir=tdir
            )
            LAST_EXEC_TIME_NS = res.exec_time_ns
        except Exception as exc:  # trace path can fail; retry without
            import traceback

            traceback.print_exc()
            print(f"[kernel] trace run failed ({exc!r}); rerunning untraced")
            res = None
    if res is None:
        res = run_bass_kernel_spmd(nc, in_maps, core_ids=list(range(B)))
    LAST_RESULTS = res

    out = np.stack([np.asarray(res.results[b]["out"]) for b in range(B)])
    attn = np.stack([np.asarray(res.results[b]["attn"]) for b in range(B)])
    return out, attn


# revision 22
# speedup vs baseline: 1.2138x; 1.0145x over previous
"""Trainium2 Bass kernel for batched multi-head attention with additive bias.

Reference computation (per batch b):
    q = (x @ Wq) * d**-0.5, k = x @ Wk, v = x @ Wv      (heads split, d=64, nh=12)
    scores = q @ k^T + attn_bias                         ([nh, N, N], bias broadcast)
    attn   = softmax(scores, axis=-1)                    (returned as output #2)
    out    = (attn @ v).merge_heads() @ Wproj            (returned as output #1)

Sharding: data-parallel over batch B=8 across the 8 NeuronCores (one batch
element per core, weights + bias replicated). No collectives needed.

Per-core dataflow (all matmuls in float32r = PE native fast fp32 mode):
  - host passes x[b]^T so qT/kT (transposed) and v (natural) come straight
    out of the PE without any on-device transpose of x.
  - scores are built per (head-pair, q-tile) in PSUM: K=64 QK^T matmuls for
    two heads packed into disjoint PE row groups, then the bias is
    accumulated into PSUM with an identity-matmul (bias in bf16 - its
    magnitude is 0.02 so bf16 rounding is ~1e-5 absolute).
  - softmax: no max-subtraction needed (scores are O(1), exp can't overflow
    in fp32). ScalarE Exp reads PSUM and accumulates the row sum for free
    (accum_out). VectorE computes reciprocals and multiplies rows by 1/sum,
    writing bf16 staging tiles (GpSimd only generates the cast-DMA
    descriptors - its compute path measured ~20x slower than DVE).
  - attn tiles are DMA'd out in 1MB chunks (SWDGE bf16->f32 cast) and also
    transposed on the PE via REGULAR matmuls against the identity (exact
    for bf16, and unlike transpose-mode they keep the HAM clock gate at
    2.4 GHz) into [k, q] layout feeding the attn @ v matmuls.
  - final projection contracts ctx^T against Wproj giving the output in
    natural [n, c] layout for a single contiguous DMA.
"""

import os

os.environ.setdefault("MYCRO_LOCAL_CACHE", "1")

import numpy as np
import ml_dtypes

B, N, C = 8, 1024, 768
NH, D = 12, 64
P = 128
F32R = None  # filled after mybir import

_CACHE = {}
LAST_EXEC_TIME_NS = None
LAST_RESULTS = None


def _build():
    from contextlib import ExitStack

    import concourse.bass as bass
    import concourse.tile as tile
    from concourse import bacc, mybir

    f32 = mybir.dt.float32
    f32r = mybir.dt.float32r
    bf16 = mybir.dt.bfloat16
    Exp = mybir.ActivationFunctionType.Exp
    mult = mybir.AluOpType.mult

    nc = bacc.Bacc(
        "TRN2", target_bir_lowering=False, debug=False, num_devices=8
    )

    xT_d = nc.dram_tensor("xT", [C, N], f32r, kind="ExternalInput")
    Wq_d = nc.dram_tensor("Wq", [C, C], f32r, kind="ExternalInput")
    Wk_d = nc.dram_tensor("Wk", [C, C], f32r, kind="ExternalInput")
    Wv_d = nc.dram_tensor("Wv", [C, C], f32r, kind="ExternalInput")
    Wp_d = nc.dram_tensor("Wproj", [C, C], f32r, kind="ExternalInput")
    bias_d = nc.dram_tensor("biasb", [N, N], bf16, kind="ExternalInput")
    out_d = nc.dram_tensor("out", [N, C], f32, kind="ExternalOutput")
    attn_d = nc.dram_tensor("attn", [NH, N, N], f32, kind="ExternalOutput")

    identb_d = nc.inline_tensor(
        np.eye(P, dtype=ml_dtypes.bfloat16), name="identb"
    )

    CT = C // P  # 6 column tiles of 128
    NT = N // P  # 8 seq tiles of 128
    NPAIR = NH // 2  # 6 head pairs; pair t occupies c' columns [t*128, t*128+128)

    with tile.TileContext(nc) as tc:
        with ExitStack() as ctx:
            pers = ctx.enter_context(tc.tile_pool(name="pers", bufs=1))
            qTs = pers.tile([P, CT, N], f32r)  # q^T, scaled by d**-0.5
            kTs = pers.tile([P, CT, N], f32r)  # k^T
            vs = pers.tile([P, NT, C], bf16)  # v natural [n, c'] (bf16 for PV)
            biass = pers.tile([P, NT, N], bf16)  # bias [q, k], q partition-tiled
            ctxT = pers.tile([P, CT, N], f32r)  # (attn@v)^T accumulated per pair
            sums = pers.tile([P, NT, NH], f32)
            recips = pers.tile([P, NT, NH], f32)
            identb = pers.tile([P, P], bf16)
            wpsb = pers.tile([P, CT, C], f32r)
            nc.sync.dma_start(
                out=wpsb[:], in_=Wp_d.ap().rearrange("(o p) f -> p o f", p=P)
            )

            # ---------------- Phase 1: QKV projections ----------------
            with ExitStack() as ph1:
                p1x = ph1.enter_context(tc.tile_pool(name="p1x", bufs=1))
                p1w = ph1.enter_context(tc.tile_pool(name="p1w", bufs=2))
                p1ps = ph1.enter_context(
                    tc.tile_pool(name="p1ps", bufs=2, space="PSUM")
                )

                xTs = p1x.tile([P, CT, N], f32r)
                xT_ap = xT_d.ap().rearrange("(o p) n -> p o n", p=P)
                nc.sync.dma_start(out=identb[:], in_=identb_d.ap())
                for k in range(CT):
                    nc.sync.dma_start(out=xTs[:, k, :], in_=xT_ap[:, k, :])
                # HAM pre-warm: the activity monitor needs ~3.4us of
                # sustained matmul work before it lifts the PE clock gate
                # from 1.2 to 2.4 GHz. Burn ~40 tiny identity matmuls into a
                # scratch PSUM tile while the input DMAs stream in, so the
                # real QKV matmuls start at full clock.
                warm = p1ps.tile([P, P], f32, tag="warm")
                for _ in range(40):
                    nc.tensor.matmul(
                        warm[:],
                        lhsT=identb[:],
                        rhs=identb[:],
                        start=True,
                        stop=True,
                        skip_group_check=True,
                    )
                wq_first = True

                for w_d, mode in ((Wq_d, "q"), (Wk_d, "k"), (Wv_d, "v")):
                    wsb = p1w.tile([P, CT, C], f32r, tag="w")
                    w_ap = w_d.ap().rearrange("(o p) f -> p o f", p=P)
                    for k in range(CT):
                        nc.sync.dma_start(out=wsb[:, k, :], in_=w_ap[:, k, :])
                    if wq_first:
                        # queued after the tensors the first matmuls need
                        wq_first = False
                        nc.sync.dma_start(
                            out=biass[:],
                            in_=bias_d.ap().rearrange("(o p) k -> p o k", p=P),
                        )
                    if mode in ("q", "k"):
                        dst = qTs if mode == "q" else kTs
                        # dst[c', n] = sum_c W[c, c'] xT[c, n]
                        for m in range(CT):
                            for nh2 in range(2):
                                ps = p1ps.tile([P, C], f32, tag="p")
                                for k in range(CT):
                                    nc.tensor.matmul(
                                        ps[:, :512],
                                        lhsT=wsb[
                                            :, k, m * P : (m + 1) * P
                                        ],
                                        rhs=xTs[
                                            :, k, nh2 * 512 : nh2 * 512 + 512
                                        ],
                                        start=(k == 0),
                                        stop=(k == CT - 1),
                                    )
                                dslice = dst[:, m, nh2 * 512 : nh2 * 512 + 512]
                                if mode == "q":
                                    nc.vector.tensor_scalar_mul(
                                        dslice, ps[:, :512], float(D**-0.5)
                                    )
                                else:
                                    nc.any.tensor_copy(
                                        out=dslice, in_=ps[:, :512]
                                    )
                    else:
                        # v[n, c'] = sum_c xT[c, n] Wv[c, c']
                        for m in range(NT):
                            ps = p1ps.tile([P, C], f32, tag="p")
                            for off, nn in ((0, 512), (512, 256)):
                                for k in range(CT):
                                    nc.tensor.matmul(
                                        ps[:, off : off + nn],
                                        lhsT=xTs[
                                            :, k, m * P : (m + 1) * P
                                        ],
                                        rhs=wsb[:, k, off : off + nn],
                                        start=(k == 0),
                                        stop=(k == CT - 1),
                                    )
                            nc.any.tensor_copy(out=vs[:, m, :], in_=ps[:])

            # ---------------- Phase 2: attention per head pair ----------------
            with ExitStack() as ph2:
                s_ps = ph2.enter_context(
                    tc.tile_pool(name="s_ps", bufs=2, space="PSUM")
                )
                tp_ps = ph2.enter_context(
                    tc.tile_pool(name="tp_ps", bufs=3, space="PSUM")
                )
                pv_ps = ph2.enter_context(
                    tc.tile_pool(name="pv_ps", bufs=1, space="PSUM")
                )
                tmpb_pool = ph2.enter_context(tc.tile_pool(name="tmpb", bufs=2))
                e_pool = ph2.enter_context(tc.tile_pool(name="e", bufs=8))
                et_pool = ph2.enter_context(tc.tile_pool(name="et", bufs=2))
                as_pool = ph2.enter_context(tc.tile_pool(name="astg", bufs=4))

                attn_ap = attn_d.ap().rearrange("h (o p) k -> h p o k", p=P)

                for t in range(NPAIR):
                    hA, hB = 2 * t, 2 * t + 1
                    tmpB = tmpb_pool.tile([64, N], f32r, tag="tmpb")
                    for blk in range(4):
                        stA = as_pool.tile([P, 2, N], bf16, tag="astg")
                        stB = as_pool.tile([P, 2, N], bf16, tag="astg")
                        eT = et_pool.tile([P, NT, 512], bf16, tag="et")
                        for i in range(2):
                            qt = 2 * blk + i
                            psA = s_ps.tile([P, N], f32, tag="s")
                            psB = s_ps.tile([P, N], f32, tag="s")
                            for kh in range(2):
                                sl = slice(kh * 512, kh * 512 + 512)
                                # two heads packed into PE row groups 0-63 / 64-127
                                nc.tensor.matmul(
                                    psA[:, sl],
                                    lhsT=qTs[
                                        0:64, t, qt * P : (qt + 1) * P
                                    ],
                                    rhs=kTs[0:64, t, sl],
                                    start=True,
                                    stop=False,
                                    tile_position=(0, 0),
                                )
                                nc.tensor.matmul(
                                    psB[:, sl],
                                    lhsT=qTs[
                                        64:128, t, qt * P : (qt + 1) * P
                                    ],
                                    rhs=kTs[64:128, t, sl],
                                    start=True,
                                    stop=False,
                                    tile_position=(64, 0),
                                )
                                # additive bias via identity matmul accumulate
                                nc.tensor.matmul(
                                    psA[:, sl],
                                    lhsT=identb[:],
                                    rhs=biass[:, qt, sl],
                                    start=False,
                                    stop=True,
                                )
                                nc.tensor.matmul(
                                    psB[:, sl],
                                    lhsT=identb[:],
                                    rhs=biass[:, qt, sl],
                                    start=False,
                                    stop=True,
                                )
                            eA = e_pool.tile([P, N], f32, tag="e")
                            eB = e_pool.tile([P, N], f32, tag="e")
                            nc.scalar.activation(
                                eA[:],
                                psA[:],
                                Exp,
                                accum_out=sums[:, qt, hA : hA + 1],
                            )
                            nc.scalar.activation(
                                eB[:],
                                psB[:],
                                Exp,
                                accum_out=sums[:, qt, hB : hB + 1],
                            )
                            nc.vector.reciprocal(
                                recips[:, qt, hA : hA + 2],
                                sums[:, qt, hA : hA + 2],
                            )
                            # normalize on DVE (fp32 SBUF tensor_scalar runs
                            # in 2x mode; GpSimd measured 14.7us/tile here -
                            # ~20x slower - so it gets no work at all)
                            nc.vector.tensor_scalar(
                                stA[:, i, :],
                                eA[:],
                                recips[:, qt, hA : hA + 1],
                                None,
                                mult,
                            )
                            nc.vector.tensor_scalar(
                                stB[:, i, :],
                                eB[:],
                                recips[:, qt, hB : hB + 1],
                                None,
                                mult,
                            )
                        # stream normalized attention to DRAM (1MB per DMA,
                        # bf16->f32 cast done by the SWDGE path)
                        nc.gpsimd.dma_start(
                            out=attn_ap[hA, :, 2 * blk : 2 * blk + 2, :],
                            in_=stA[:],
                        )
                        nc.gpsimd.dma_start(
                            out=attn_ap[hB, :, 2 * blk : 2 * blk + 2, :],
                            in_=stB[:],
                        )
                        # transpose the 2 q-tiles x 2 heads into [k, q]
                        # layout via REGULAR matmuls against the identity
                        # (out = st_tile.T @ I, exact for bf16): unlike
                        # transpose-mode these count as matmul activity for
                        # the HAM clock gate, keeping the PE at 2.4 GHz.
                        # (A DMA-xbar version passed CoreSim but produced
                        # garbage on hardware - known sim/HW divergence.)
                        for kt in range(NT):
                            tp = tp_ps.tile([P, 512], f32, tag="tp")
                            for j, st in enumerate((stA, stA, stB, stB)):
                                i = j % 2
                                nc.tensor.matmul(
                                    tp[:, j * P : (j + 1) * P],
                                    lhsT=st[:, i, kt * P : (kt + 1) * P],
                                    rhs=identb[:],
                                    start=True,
                                    stop=True,
                                    skip_group_check=True,
                                )
                            if kt % 2 == 0:
                                nc.scalar.copy(out=eT[:, kt, :], in_=tp[:])
                            else:
                                nc.vector.tensor_copy(
                                    out=eT[:, kt, :], in_=tp[:]
                                )
                        # attn @ v for this 256-wide q block: head A into
                        # cols 0:256, head B into cols 256:512 of one PSUM
                        # bank (partitions 0-63); accumulate over k-tiles.
                        # One start=True clears the bank's has_written bits;
                        # every later matmul overwrites untouched elements
                        # and accumulates written ones, so head B needs no
                        # start flag of its own.
                        pv = pv_ps.tile([64, 512], f32, tag="pv")
                        for kt in range(NT):
                            nc.tensor.matmul(
                                pv[:, 0:256],
                                lhsT=vs[:, kt, t * P : t * P + 64],
                                rhs=eT[:, kt, 0:256],
                                start=(kt == 0),
                                stop=False,
                                skip_group_check=True,
                            )
                            nc.tensor.matmul(
                                pv[:, 256:512],
                                lhsT=vs[:, kt, t * P + 64 : t * P + 128],
                                rhs=eT[:, kt, 256:512],
                                start=False,
                                stop=(kt == NT - 1),
                                skip_group_check=True,
                            )
                        bsl = slice(blk * 256, blk * 256 + 256)
                        nc.any.tensor_copy(
                            out=ctxT[0:64, t, bsl], in_=pv[:, 0:256]
                        )
                        nc.any.tensor_copy(
                            out=tmpB[:, bsl], in_=pv[:, 256:512]
                        )
                    # head B context lives at partitions 0-63; DMA shifts it
                    # to partitions 64-127 of ctxT (engines can't cross
                    # partitions, DMA can).
                    nc.sync.dma_start(out=ctxT[64:128, t, :], in_=tmpB[:])

            # ---------------- Phase 3: output projection ----------------
            with ExitStack() as ph3:
                p3 = ph3.enter_context(tc.tile_pool(name="p3", bufs=1))
                p3ps = ph3.enter_context(
                    tc.tile_pool(name="p3ps", bufs=2, space="PSUM")
                )
                outst = p3.tile([P, NT, C], f32)
                for m in range(NT):
                    ps = p3ps.tile([P, C], f32, tag="pp")
                    for off, nn in ((0, 512), (512, 256)):
                        for k in range(CT):
                            nc.tensor.matmul(
                                ps[:, off : off + nn],
                                lhsT=ctxT[:, k, m * P : (m + 1) * P],
                                rhs=wpsb[:, k, off : off + nn],
                                start=(k == 0),
                                stop=(k == CT - 1),
                            )
                    nc.any.tensor_copy(out=outst[:, m, :], in_=ps[:])
                nc.sync.dma_start(
                    out=out_d.ap().rearrange("(o p) c -> p o c", p=P),
                    in_=outst[:],
                )

    nc.compile()
    return nc


def _install_axon_ntff_hook():
    """Provide antenv.axon_hooks (absent in this image) so that
    run_bass_kernel_spmd(trace=True) can capture NTFF profiles through
    the axon sidechannel. Mirrors trn_agent_boot.trn_boot."""
    import sys
    import types
    import ctypes
    import contextlib

    if "antenv.axon_hooks" in sys.modules:
        return
    so_path = "/opt/axon/libaxon_pjrt.so"
    lib = ctypes.CDLL(so_path)
    if not hasattr(lib, "axon_start_nrt_profile"):
        return
    lib.axon_start_nrt_profile.argtypes = [
        ctypes.POINTER(ctypes.c_int64),
        ctypes.c_size_t,
    ]
    lib.axon_start_nrt_profile.restype = ctypes.c_int64
    lib.axon_stop_nrt_profile.argtypes = [ctypes.c_char_p]
    lib.axon_stop_nrt_profile.restype = ctypes.c_int64

    @contextlib.contextmanager
    def _hook(output_dir, device_ids):
        import jax

        jax.devices()
        if device_ids:
            ids = (ctypes.c_int64 * len(device_ids))(*device_ids)
            rc = lib.axon_start_nrt_profile(ids, len(device_ids))
        else:
            rc = lib.axon_start_nrt_profile(None, 0)
        if rc != 0:
            raise RuntimeError(f"axon_start_nrt_profile rc={rc}")
        try:
            yield
        finally:
            n = lib.axon_stop_nrt_profile(str(output_dir).encode())
            print(f"[kernel] ntff profile: {n} file(s) -> {output_dir}")

    mod = types.ModuleType("antenv.axon_hooks")
    mod.get_axon_ntff_profile_hook = lambda: _hook
    mod.set_axon_ntff_profile_hook = lambda h: None
    sys.modules["antenv.axon_hooks"] = mod


def kernel(x, Wq, Wk, Wv, Wproj, attn_bias):
    global LAST_EXEC_TIME_NS, LAST_RESULTS
    from concourse.bass_utils import run_bass_kernel_spmd

    if "nc" not in _CACHE:
        _CACHE["nc"] = _build()
    nc = _CACHE["nc"]

    x = np.ascontiguousarray(np.asarray(x, dtype=np.float32))
    bias_bf = np.ascontiguousarray(
        np.asarray(attn_bias, dtype=np.float32).astype(ml_dtypes.bfloat16)
    )
    Wq = np.ascontiguousarray(np.asarray(Wq, dtype=np.float32))
    Wk = np.ascontiguousarray(np.asarray(Wk, dtype=np.float32))
    Wv = np.ascontiguousarray(np.asarray(Wv, dtype=np.float32))
    Wproj = np.ascontiguousarray(np.asarray(Wproj, dtype=np.float32))

    in_maps = [
        {
            "xT": np.ascontiguousarray(x[b].T),
            "Wq": Wq,
            "Wk": Wk,
            "Wv": Wv,
            "Wproj": Wproj,
            "biasb": bias_bf,
        }
        for b in range(B)
    ]

    trace = os.environ.get("KERNEL_PROFILE", "0") == "1"
    res = None
    if trace:
        try:
            _install_axon_ntff_hook()
            tdir = os.environ.get("KERNEL_TRACE_DIR") or None
            res = run_bass_kernel_spmd(
                nc, in_maps, core_ids=list(range(B)), trace=True, tmpdir=tdir
            )
            LAST_EXEC_TIME_NS = res.exec_time_ns
        except Exception as exc:  # trace path can fail; retry without
            import traceback

            traceback.print_exc()
            print(f"[kernel] trace run failed ({exc!r}); rerunning untraced")
            res = None
    if res is None:
        res = run_bass_kernel_spmd(nc, in_maps, core_ids=list(range(B)))
    LAST_RESULTS = res

    out = np.stack([np.asarray(res.results[b]["out"]) for b in range(B)])
    attn = np.stack([np.asarray(res.results[b]["attn"]) for b in range(B)])
    return out, attn


# revision 23
# speedup vs baseline: 1.2157x; 1.0016x over previous
"""Trainium2 Bass kernel for batched multi-head attention with additive bias.

Reference computation (per batch b):
    q = (x @ Wq) * d**-0.5, k = x @ Wk, v = x @ Wv      (heads split, d=64, nh=12)
    scores = q @ k^T + attn_bias                         ([nh, N, N], bias broadcast)
    attn   = softmax(scores, axis=-1)                    (returned as output #2)
    out    = (attn @ v).merge_heads() @ Wproj            (returned as output #1)

Sharding: data-parallel over batch B=8 across the 8 NeuronCores (one batch
element per core, weights + bias replicated). No collectives needed.

Per-core dataflow (all matmuls in float32r = PE native fast fp32 mode):
  - host passes x[b]^T so qT/kT (transposed) and v (natural) come straight
    out of the PE without any on-device transpose of x.
  - scores are built per (head-pair, q-tile) in PSUM: K=64 QK^T matmuls for
    two heads packed into disjoint PE row groups, then the bias is
    accumulated into PSUM with an identity-matmul (bias in bf16 - its
    magnitude is 0.02 so bf16 rounding is ~1e-5 absolute).
  - softmax: no max-subtraction needed (scores are O(1), exp can't overflow
    in fp32). ScalarE Exp reads PSUM and accumulates the row sum for free
    (accum_out). VectorE computes reciprocals and multiplies rows by 1/sum,
    writing bf16 staging tiles (GpSimd only generates the cast-DMA
    descriptors - its compute path measured ~20x slower than DVE).
  - attn tiles are DMA'd out in 1MB chunks (SWDGE bf16->f32 cast) and also
    transposed on the PE via REGULAR matmuls against the identity (exact
    for bf16, and unlike transpose-mode they keep the HAM clock gate at
    2.4 GHz) into [k, q] layout feeding the attn @ v matmuls.
  - final projection contracts ctx^T against Wproj giving the output in
    natural [n, c] layout for a single contiguous DMA.
"""

import os

os.environ.setdefault("MYCRO_LOCAL_CACHE", "1")

import numpy as np
import ml_dtypes

B, N, C = 8, 1024, 768
NH, D = 12, 64
P = 128
F32R = None  # filled after mybir import

_CACHE = {}
LAST_EXEC_TIME_NS = None
LAST_RESULTS = None


def _build():
    from contextlib import ExitStack

    import concourse.bass as bass
    import concourse.tile as tile
    from concourse import bacc, mybir

    f32 = mybir.dt.float32
    f32r = mybir.dt.float32r
    bf16 = mybir.dt.bfloat16
    Exp = mybir.ActivationFunctionType.Exp
    mult = mybir.AluOpType.mult

    nc = bacc.Bacc(
        "TRN2", target_bir_lowering=False, debug=False, num_devices=8
    )

    xT_d = nc.dram_tensor("xT", [C, N], f32r, kind="ExternalInput")
    Wq_d = nc.dram_tensor("Wq", [C, C], f32r, kind="ExternalInput")
    Wk_d = nc.dram_tensor("Wk", [C, C], f32r, kind="ExternalInput")
    Wv_d = nc.dram_tensor("Wv", [C, C], f32r, kind="ExternalInput")
    Wp_d = nc.dram_tensor("Wproj", [C, C], f32r, kind="ExternalInput")
    bias_d = nc.dram_tensor("biasb", [N, N], bf16, kind="ExternalInput")
    out_d = nc.dram_tensor("out", [N, C], f32, kind="ExternalOutput")
    attn_d = nc.dram_tensor("attn", [NH, N, N], f32, kind="ExternalOutput")

    identb_d = nc.inline_tensor(
        np.eye(P, dtype=ml_dtypes.bfloat16), name="identb"
    )

    CT = C // P  # 6 column tiles of 128
    NT = N // P  # 8 seq tiles of 128
    NPAIR = NH // 2  # 6 head pairs; pair t occupies c' columns [t*128, t*128+128)

    with tile.TileContext(nc) as tc:
        with ExitStack() as ctx:
            pers = ctx.enter_context(tc.tile_pool(name="pers", bufs=1))
            qTs = pers.tile([P, CT, N], f32r)  # q^T, scaled by d**-0.5
            kTs = pers.tile([P, CT, N], f32r)  # k^T
            vs = pers.tile([P, NT, C], bf16)  # v natural [n, c'] (bf16 for PV)
            biass = pers.tile([P, NT, N], bf16)  # bias [q, k], q partition-tiled
            ctxT = pers.tile([P, CT, N], f32r)  # (attn@v)^T accumulated per pair
            sums = pers.tile([P, NT, NH], f32)
            recips = pers.tile([P, NT, NH], f32)
            identb = pers.tile([P, P], bf16)
            wpsb = pers.tile([P, CT, C], f32r)
            nc.sync.dma_start(
                out=wpsb[:], in_=Wp_d.ap().rearrange("(o p) f -> p o f", p=P)
            )

            # ---------------- Phase 1: QKV projections ----------------
            with ExitStack() as ph1:
                p1x = ph1.enter_context(tc.tile_pool(name="p1x", bufs=1))
                p1w = ph1.enter_context(tc.tile_pool(name="p1w", bufs=2))
                p1ps = ph1.enter_context(
                    tc.tile_pool(name="p1ps", bufs=2, space="PSUM")
                )

                xTs = p1x.tile([P, CT, N], f32r)
                xT_ap = xT_d.ap().rearrange("(o p) n -> p o n", p=P)
                nc.sync.dma_start(out=identb[:], in_=identb_d.ap())
                for k in range(CT):
                    nc.sync.dma_start(out=xTs[:, k, :], in_=xT_ap[:, k, :])
                # HAM pre-warm: the activity monitor needs ~3.4us of
                # sustained matmul work before it lifts the PE clock gate
                # from 1.2 to 2.4 GHz. Burn ~40 tiny identity matmuls into a
                # scratch PSUM tile while the input DMAs stream in, so the
                # real QKV matmuls start at full clock.
                warm = p1ps.tile([P, P], f32, tag="warm")
                for _ in range(130):
                    nc.tensor.matmul(
                        warm[:],
                        lhsT=identb[:],
                        rhs=identb[:],
                        start=True,
                        stop=True,
                        skip_group_check=True,
                    )
                wq_first = True

                for w_d, mode in ((Wq_d, "q"), (Wk_d, "k"), (Wv_d, "v")):
                    wsb = p1w.tile([P, CT, C], f32r, tag="w")
                    w_ap = w_d.ap().rearrange("(o p) f -> p o f", p=P)
                    for k in range(CT):
                        nc.sync.dma_start(out=wsb[:, k, :], in_=w_ap[:, k, :])
                    if wq_first:
                        # queued after the tensors the first matmuls need
                        wq_first = False
                        nc.sync.dma_start(
                            out=biass[:],
                            in_=bias_d.ap().rearrange("(o p) k -> p o k", p=P),
                        )
                    if mode in ("q", "k"):
                        dst = qTs if mode == "q" else kTs
                        # dst[c', n] = sum_c W[c, c'] xT[c, n]
                        for m in range(CT):
                            for nh2 in range(2):
                                ps = p1ps.tile([P, C], f32, tag="p")
                                for k in range(CT):
                                    nc.tensor.matmul(
                                        ps[:, :512],
                                        lhsT=wsb[
                                            :, k, m * P : (m + 1) * P
                                        ],
                                        rhs=xTs[
                                            :, k, nh2 * 512 : nh2 * 512 + 512
                                        ],
                                        start=(k == 0),
                                        stop=(k == CT - 1),
                                    )
                                dslice = dst[:, m, nh2 * 512 : nh2 * 512 + 512]
                                if mode == "q":
                                    nc.vector.tensor_scalar_mul(
                                        dslice, ps[:, :512], float(D**-0.5)
                                    )
                                else:
                                    nc.any.tensor_copy(
                                        out=dslice, in_=ps[:, :512]
                                    )
                    else:
                        # v[n, c'] = sum_c xT[c, n] Wv[c, c']
                        for m in range(NT):
                            ps = p1ps.tile([P, C], f32, tag="p")
                            for off, nn in ((0, 512), (512, 256)):
                                for k in range(CT):
                                    nc.tensor.matmul(
                                        ps[:, off : off + nn],
                                        lhsT=xTs[
                                            :, k, m * P : (m + 1) * P
                                        ],
                                        rhs=wsb[:, k, off : off + nn],
                                        start=(k == 0),
                                        stop=(k == CT - 1),
                                    )
                            nc.any.tensor_copy(out=vs[:, m, :], in_=ps[:])

            # ---------------- Phase 2: attention per head pair ----------------
            with ExitStack() as ph2:
                s_ps = ph2.enter_context(
                    tc.tile_pool(name="s_ps", bufs=2, space="PSUM")
                )
                tp_ps = ph2.enter_context(
                    tc.tile_pool(name="tp_ps", bufs=3, space="PSUM")
                )
                pv_ps = ph2.enter_context(
                    tc.tile_pool(name="pv_ps", bufs=1, space="PSUM")
                )
                tmpb_pool = ph2.enter_context(tc.tile_pool(name="tmpb", bufs=2))
                e_pool = ph2.enter_context(tc.tile_pool(name="e", bufs=8))
                et_pool = ph2.enter_context(tc.tile_pool(name="et", bufs=2))
                as_pool = ph2.enter_context(tc.tile_pool(name="astg", bufs=4))

                attn_ap = attn_d.ap().rearrange("h (o p) k -> h p o k", p=P)

                for t in range(NPAIR):
                    hA, hB = 2 * t, 2 * t + 1
                    tmpB = tmpb_pool.tile([64, N], f32r, tag="tmpb")
                    for blk in range(4):
                        stA = as_pool.tile([P, 2, N], bf16, tag="astg")
                        stB = as_pool.tile([P, 2, N], bf16, tag="astg")
                        eT = et_pool.tile([P, NT, 512], bf16, tag="et")
                        for i in range(2):
                            qt = 2 * blk + i
                            psA = s_ps.tile([P, N], f32, tag="s")
                            psB = s_ps.tile([P, N], f32, tag="s")
                            for kh in range(2):
                                sl = slice(kh * 512, kh * 512 + 512)
                                # two heads packed into PE row groups 0-63 / 64-127
                                nc.tensor.matmul(
                                    psA[:, sl],
                                    lhsT=qTs[
                                        0:64, t, qt * P : (qt + 1) * P
                                    ],
                                    rhs=kTs[0:64, t, sl],
                                    start=True,
                                    stop=False,
                                    tile_position=(0, 0),
                                )
                                nc.tensor.matmul(
                                    psB[:, sl],
                                    lhsT=qTs[
                                        64:128, t, qt * P : (qt + 1) * P
                                    ],
                                    rhs=kTs[64:128, t, sl],
                                    start=True,
                                    stop=False,
                                    tile_position=(64, 0),
                                )
                                # additive bias via identity matmul accumulate
                                nc.tensor.matmul(
                                    psA[:, sl],
                                    lhsT=identb[:],
                                    rhs=biass[:, qt, sl],
                                    start=False,
                                    stop=True,
                                )
                                nc.tensor.matmul(
                                    psB[:, sl],
                                    lhsT=identb[:],
                                    rhs=biass[:, qt, sl],
                                    start=False,
                                    stop=True,
                                )
                            eA = e_pool.tile([P, N], f32, tag="e")
                            eB = e_pool.tile([P, N], f32, tag="e")
                            nc.scalar.activation(
                                eA[:],
                                psA[:],
                                Exp,
                                accum_out=sums[:, qt, hA : hA + 1],
                            )
                            nc.scalar.activation(
                                eB[:],
                                psB[:],
                                Exp,
                                accum_out=sums[:, qt, hB : hB + 1],
                            )
                            nc.vector.reciprocal(
                                recips[:, qt, hA : hA + 2],
                                sums[:, qt, hA : hA + 2],
                            )
                            # normalize on DVE (fp32 SBUF tensor_scalar runs
                            # in 2x mode; GpSimd measured 14.7us/tile here -
                            # ~20x slower - so it gets no work at all)
                            nc.vector.tensor_scalar(
                                stA[:, i, :],
                                eA[:],
                                recips[:, qt, hA : hA + 1],
                                None,
                                mult,
                            )
                            nc.vector.tensor_scalar(
                                stB[:, i, :],
                                eB[:],
                                recips[:, qt, hB : hB + 1],
                                None,
                                mult,
                            )
                        # stream normalized attention to DRAM (1MB per DMA,
                        # bf16->f32 cast done by the SWDGE path)
                        nc.gpsimd.dma_start(
                            out=attn_ap[hA, :, 2 * blk : 2 * blk + 2, :],
                            in_=stA[:],
                        )
                        nc.gpsimd.dma_start(
                            out=attn_ap[hB, :, 2 * blk : 2 * blk + 2, :],
                            in_=stB[:],
                        )
                        # transpose the 2 q-tiles x 2 heads into [k, q]
                        # layout via REGULAR matmuls against the identity
                        # (out = st_tile.T @ I, exact for bf16): unlike
                        # transpose-mode these count as matmul activity for
                        # the HAM clock gate, keeping the PE at 2.4 GHz.
                        # (A DMA-xbar version passed CoreSim but produced
                        # garbage on hardware - known sim/HW divergence.)
                        for kt in range(NT):
                            tp = tp_ps.tile([P, 512], f32, tag="tp")
                            for j, st in enumerate((stA, stA, stB, stB)):
                                i = j % 2
                                nc.tensor.matmul(
                                    tp[:, j * P : (j + 1) * P],
                                    lhsT=st[:, i, kt * P : (kt + 1) * P],
                                    rhs=identb[:],
                                    start=True,
                                    stop=True,
                                    skip_group_check=True,
                                )
                            if kt % 2 == 0:
                                nc.scalar.copy(out=eT[:, kt, :], in_=tp[:])
                            else:
                                nc.vector.tensor_copy(
                                    out=eT[:, kt, :], in_=tp[:]
                                )
                        # attn @ v for this 256-wide q block: head A into
                        # cols 0:256, head B into cols 256:512 of one PSUM
                        # bank (partitions 0-63); accumulate over k-tiles.
                        # One start=True clears the bank's has_written bits;
                        # every later matmul overwrites untouched elements
                        # and accumulates written ones, so head B needs no
                        # start flag of its own.
                        pv = pv_ps.tile([64, 512], f32, tag="pv")
                        for kt in range(NT):
                            nc.tensor.matmul(
                                pv[:, 0:256],
                                lhsT=vs[:, kt, t * P : t * P + 64],
                                rhs=eT[:, kt, 0:256],
                                start=(kt == 0),
                                stop=False,
                                skip_group_check=True,
                            )
                            nc.tensor.matmul(
                                pv[:, 256:512],
                                lhsT=vs[:, kt, t * P + 64 : t * P + 128],
                                rhs=eT[:, kt, 256:512],
                                start=False,
                                stop=(kt == NT - 1),
                                skip_group_check=True,
                            )
                        bsl = slice(blk * 256, blk * 256 + 256)
                        nc.any.tensor_copy(
                            out=ctxT[0:64, t, bsl], in_=pv[:, 0:256]
                        )
                        nc.any.tensor_copy(
                            out=tmpB[:, bsl], in_=pv[:, 256:512]
                        )
                        # head B context lives at partitions 0-63; DMA
                        # shifts it to partitions 64-127 of ctxT (engines
                        # can't cross partitions, DMA can). Per-block so the
                        # last pair's shift overlaps its own attention work.
                        nc.sync.dma_start(
                            out=ctxT[64:128, t, bsl], in_=tmpB[:, bsl]
                        )

            # ---------------- Phase 3: output projection ----------------
            with ExitStack() as ph3:
                p3 = ph3.enter_context(tc.tile_pool(name="p3", bufs=1))
                p3ps = ph3.enter_context(
                    tc.tile_pool(name="p3ps", bufs=2, space="PSUM")
                )
                outst = p3.tile([P, NT, C], f32)
                for m in range(NT):
                    ps = p3ps.tile([P, C], f32, tag="pp")
                    for off, nn in ((0, 512), (512, 256)):
                        for k in range(CT):
                            nc.tensor.matmul(
                                ps[:, off : off + nn],
                                lhsT=ctxT[:, k, m * P : (m + 1) * P],
                                rhs=wpsb[:, k, off : off + nn],
                                start=(k == 0),
                                stop=(k == CT - 1),
                            )
                    nc.any.tensor_copy(out=outst[:, m, :], in_=ps[:])
                nc.sync.dma_start(
                    out=out_d.ap().rearrange("(o p) c -> p o c", p=P),
                    in_=outst[:],
                )

    nc.compile()
    return nc


def _install_axon_ntff_hook():
    """Provide antenv.axon_hooks (absent in this image) so that
    run_bass_kernel_spmd(trace=True) can capture NTFF profiles through
    the axon sidechannel. Mirrors trn_agent_boot.trn_boot."""
    import sys
    import types
    import ctypes
    import contextlib

    if "antenv.axon_hooks" in sys.modules:
        return
    so_path = "/opt/axon/libaxon_pjrt.so"
    lib = ctypes.CDLL(so_path)
    if not hasattr(lib, "axon_start_nrt_profile"):
        return
    lib.axon_start_nrt_profile.argtypes = [
        ctypes.POINTER(ctypes.c_int64),
        ctypes.c_size_t,
    ]
    lib.axon_start_nrt_profile.restype = ctypes.c_int64
    lib.axon_stop_nrt_profile.argtypes = [ctypes.c_char_p]
    lib.axon_stop_nrt_profile.restype = ctypes.c_int64

    @contextlib.contextmanager
    def _hook(output_dir, device_ids):
        import jax

        jax.devices()
        if device_ids:
            ids = (ctypes.c_int64 * len(device_ids))(*device_ids)
            rc = lib.axon_start_nrt_profile(ids, len(device_ids))
        else:
            rc = lib.axon_start_nrt_profile(None, 0)
        if rc != 0:
            raise RuntimeError(f"axon_start_nrt_profile rc={rc}")
        try:
            yield
        finally:
            n = lib.axon_stop_nrt_profile(str(output_dir).encode())
            print(f"[kernel] ntff profile: {n} file(s) -> {output_dir}")

    mod = types.ModuleType("antenv.axon_hooks")
    mod.get_axon_ntff_profile_hook = lambda: _hook
    mod.set_axon_ntff_profile_hook = lambda h: None
    sys.modules["antenv.axon_hooks"] = mod


def kernel(x, Wq, Wk, Wv, Wproj, attn_bias):
    global LAST_EXEC_TIME_NS, LAST_RESULTS
    from concourse.bass_utils import run_bass_kernel_spmd

    if "nc" not in _CACHE:
        _CACHE["nc"] = _build()
    nc = _CACHE["nc"]

    x = np.ascontiguousarray(np.asarray(x, dtype=np.float32))
    bias_bf = np.ascontiguousarray(
        np.asarray(attn_bias, dtype=np.float32).astype(ml_dtypes.bfloat16)
    )
    Wq = np.ascontiguousarray(np.asarray(Wq, dtype=np.float32))
    Wk = np.ascontiguousarray(np.asarray(Wk, dtype=np.float32))
    Wv = np.ascontiguousarray(np.asarray(Wv, dtype=np.float32))
    Wproj = np.ascontiguousarray(np.asarray(Wproj, dtype=np.float32))

    in_maps = [
        {
            "xT": np.ascontiguousarray(x[b].T),
            "Wq": Wq,
            "Wk": Wk,
            "Wv": Wv,
            "Wproj": Wproj,
            "biasb": bias_bf,
        }
        for b in range(B)
    ]

    trace = os.environ.get("KERNEL_PROFILE", "0") == "1"
    res = None
    if trace:
        try:
            _install_axon_ntff_hook()
            tdir = os.environ.get("KERNEL_TRACE_DIR") or None
            res = run_bass_kernel_spmd(
                nc, in_maps, core_ids=list(range(B)), trace=True, tmpdir=tdir
            )
            LAST_EXEC_TIME_NS = res.exec_time_ns
        except Exception as exc:  # trace path can fail; retry without
            import traceback

            traceback.print_exc()
            print(f"[kernel] trace run failed ({exc!r}); rerunning untraced")
            res = None
    if res is None:
        res = run_bass_kernel_spmd(nc, in_maps, core_ids=list(range(B)))
    LAST_RESULTS = res

    out = np.stack([np.asarray(res.results[b]["out"]) for b in range(B)])
    attn = np.stack([np.asarray(res.results[b]["attn"]) for b in range(B)])
    return out, attn


# revision 24
# speedup vs baseline: 1.2674x; 1.0425x over previous
"""Trainium2 Bass kernel for batched multi-head attention with additive bias.

Reference computation (per batch b):
    q = (x @ Wq) * d**-0.5, k = x @ Wk, v = x @ Wv      (heads split, d=64, nh=12)
    scores = q @ k^T + attn_bias                         ([nh, N, N], bias broadcast)
    attn   = softmax(scores, axis=-1)                    (returned as output #2)
    out    = (attn @ v).merge_heads() @ Wproj            (returned as output #1)

Sharding: data-parallel over batch B=8 across the 8 NeuronCores (one batch
element per core, weights + bias replicated). No collectives needed.

Per-core dataflow (all matmuls in float32r = PE native fast fp32 mode):
  - host passes x[b]^T so qT/kT (transposed) and v (natural) come straight
    out of the PE without any on-device transpose of x.
  - scores are built per (head-pair, q-tile) in PSUM: K=64 QK^T matmuls for
    two heads packed into disjoint PE row groups, then the bias is
    accumulated into PSUM with an identity-matmul (bias in bf16 - its
    magnitude is 0.02 so bf16 rounding is ~1e-5 absolute).
  - softmax: no max-subtraction needed (scores are O(1), exp can't overflow
    in fp32). ScalarE Exp reads PSUM and accumulates the row sum for free
    (accum_out). VectorE computes reciprocals and multiplies rows by 1/sum,
    writing bf16 staging tiles (GpSimd only generates the cast-DMA
    descriptors - its compute path measured ~20x slower than DVE).
  - attn tiles are DMA'd out in 1MB chunks (SWDGE bf16->f32 cast) and also
    transposed on the PE via REGULAR matmuls against the identity (exact
    for bf16, and unlike transpose-mode they keep the HAM clock gate at
    2.4 GHz) into [k, q] layout feeding the attn @ v matmuls.
  - final projection contracts ctx^T against Wproj giving the output in
    natural [n, c] layout for a single contiguous DMA.
"""

import os

os.environ.setdefault("MYCRO_LOCAL_CACHE", "1")

import numpy as np
import ml_dtypes

B, N, C = 8, 1024, 768
NH, D = 12, 64
P = 128
F32R = None  # filled after mybir import

_CACHE = {}
LAST_EXEC_TIME_NS = None
LAST_RESULTS = None


def _build():
    from contextlib import ExitStack

    import concourse.bass as bass
    import concourse.tile as tile
    from concourse import bacc, mybir

    f32 = mybir.dt.float32
    f32r = mybir.dt.float32r
    bf16 = mybir.dt.bfloat16
    Exp = mybir.ActivationFunctionType.Exp
    mult = mybir.AluOpType.mult

    nc = bacc.Bacc(
        "TRN2", target_bir_lowering=False, debug=False, num_devices=8
    )

    xT_d = nc.dram_tensor("xT", [C, N], f32r, kind="ExternalInput")
    Wq_d = nc.dram_tensor("Wq", [C, C], f32r, kind="ExternalInput")
    Wk_d = nc.dram_tensor("Wk", [C, C], f32r, kind="ExternalInput")
    Wv_d = nc.dram_tensor("Wv", [C, C], f32r, kind="ExternalInput")
    Wp_d = nc.dram_tensor("Wproj", [C, C], f32r, kind="ExternalInput")
    bias_d = nc.dram_tensor("biasb", [N, N], bf16, kind="ExternalInput")
    out_d = nc.dram_tensor("out", [N, C], f32, kind="ExternalOutput")
    attn_d = nc.dram_tensor("attn", [NH, N, N], f32, kind="ExternalOutput")

    identb_d = nc.inline_tensor(
        np.eye(P, dtype=ml_dtypes.bfloat16), name="identb"
    )

    CT = C // P  # 6 column tiles of 128
    NT = N // P  # 8 seq tiles of 128
    NPAIR = NH // 2  # 6 head pairs; pair t occupies c' columns [t*128, t*128+128)

    with tile.TileContext(nc) as tc:
        with ExitStack() as ctx:
            pers = ctx.enter_context(tc.tile_pool(name="pers", bufs=1))
            qTs = pers.tile([P, CT, N], f32r)  # q^T, scaled by d**-0.5
            kTs = pers.tile([P, CT, N], f32r)  # k^T
            vs = pers.tile([P, NT, C], bf16)  # v natural [n, c'] (bf16 for PV)
            biass = pers.tile([P, NT, N], bf16)  # bias [q, k], q partition-tiled
            ctxT = pers.tile([P, CT, N], f32r)  # (attn@v)^T accumulated per pair
            sums = pers.tile([P, NT, NH], f32)
            recips = pers.tile([P, NT, NH], f32)
            identb = pers.tile([P, P], bf16)
            wpsb = pers.tile([P, CT, C], f32r)

            # ---------------- Phase 1: QKV projections ----------------
            with ExitStack() as ph1:
                p1x = ph1.enter_context(tc.tile_pool(name="p1x", bufs=1))
                p1w = ph1.enter_context(tc.tile_pool(name="p1w", bufs=2))
                p1ps = ph1.enter_context(
                    tc.tile_pool(name="p1ps", bufs=2, space="PSUM")
                )

                xTs = p1x.tile([P, CT, N], f32r)
                xT_ap = xT_d.ap().rearrange("(o p) n -> p o n", p=P)
                nc.sync.dma_start(out=identb[:], in_=identb_d.ap())
                for k in range(CT):
                    nc.sync.dma_start(out=xTs[:, k, :], in_=xT_ap[:, k, :])
                # HAM pre-warm: the activity monitor needs ~3.4us of
                # sustained matmul work before it lifts the PE clock gate
                # from 1.2 to 2.4 GHz. Burn ~40 tiny identity matmuls into a
                # scratch PSUM tile while the input DMAs stream in, so the
                # real QKV matmuls start at full clock.
                warm = p1ps.tile([P, P], f32, tag="warm")
                for _ in range(130):
                    nc.tensor.matmul(
                        warm[:],
                        lhsT=identb[:],
                        rhs=identb[:],
                        start=True,
                        stop=True,
                        skip_group_check=True,
                    )
                wq_first = True

                for w_d, mode in ((Wq_d, "q"), (Wk_d, "k"), (Wv_d, "v")):
                    wsb = p1w.tile([P, CT, C], f32r, tag="w")
                    w_ap = w_d.ap().rearrange("(o p) f -> p o f", p=P)
                    for k in range(CT):
                        nc.sync.dma_start(out=wsb[:, k, :], in_=w_ap[:, k, :])
                    if wq_first:
                        # queued after the tensors the first matmuls need
                        wq_first = False
                        nc.sync.dma_start(
                            out=biass[:],
                            in_=bias_d.ap().rearrange("(o p) k -> p o k", p=P),
                        )
                        nc.sync.dma_start(
                            out=wpsb[:],
                            in_=Wp_d.ap().rearrange("(o p) f -> p o f", p=P),
                        )
                    if mode in ("q", "k"):
                        dst = qTs if mode == "q" else kTs
                        # dst[c', n] = sum_c W[c, c'] xT[c, n]
                        for m in range(CT):
                            for nh2 in range(2):
                                ps = p1ps.tile([P, C], f32, tag="p")
                                for k in range(CT):
                                    nc.tensor.matmul(
                                        ps[:, :512],
                                        lhsT=wsb[
                                            :, k, m * P : (m + 1) * P
                                        ],
                                        rhs=xTs[
                                            :, k, nh2 * 512 : nh2 * 512 + 512
                                        ],
                                        start=(k == 0),
                                        stop=(k == CT - 1),
                                    )
                                dslice = dst[:, m, nh2 * 512 : nh2 * 512 + 512]
                                if mode == "q":
                                    nc.vector.tensor_scalar_mul(
                                        dslice, ps[:, :512], float(D**-0.5)
                                    )
                                else:
                                    nc.any.tensor_copy(
                                        out=dslice, in_=ps[:, :512]
                                    )
                    else:
                        # v[n, c'] = sum_c xT[c, n] Wv[c, c']
                        for m in range(NT):
                            ps = p1ps.tile([P, C], f32, tag="p")
                            for off, nn in ((0, 512), (512, 256)):
                                for k in range(CT):
                                    nc.tensor.matmul(
                                        ps[:, off : off + nn],
                                        lhsT=xTs[
                                            :, k, m * P : (m + 1) * P
                                        ],
                                        rhs=wsb[:, k, off : off + nn],
                                        start=(k == 0),
                                        stop=(k == CT - 1),
                                    )
                            nc.any.tensor_copy(out=vs[:, m, :], in_=ps[:])

            # ---------------- Phase 2: attention per head pair ----------------
            with ExitStack() as ph2:
                s_ps = ph2.enter_context(
                    tc.tile_pool(name="s_ps", bufs=2, space="PSUM")
                )
                tp_ps = ph2.enter_context(
                    tc.tile_pool(name="tp_ps", bufs=3, space="PSUM")
                )
                pv_ps = ph2.enter_context(
                    tc.tile_pool(name="pv_ps", bufs=1, space="PSUM")
                )
                tmpb_pool = ph2.enter_context(tc.tile_pool(name="tmpb", bufs=2))
                e_pool = ph2.enter_context(tc.tile_pool(name="e", bufs=8))
                et_pool = ph2.enter_context(tc.tile_pool(name="et", bufs=2))
                as_pool = ph2.enter_context(tc.tile_pool(name="astg", bufs=4))

                attn_ap = attn_d.ap().rearrange("h (o p) k -> h p o k", p=P)

                for t in range(NPAIR):
                    hA, hB = 2 * t, 2 * t + 1
                    tmpB = tmpb_pool.tile([64, N], f32r, tag="tmpb")
                    for blk in range(4):
                        stA = as_pool.tile([P, 2, N], bf16, tag="astg")
                        stB = as_pool.tile([P, 2, N], bf16, tag="astg")
                        eT = et_pool.tile([P, NT, 512], bf16, tag="et")
                        for i in range(2):
                            qt = 2 * blk + i
                            psA = s_ps.tile([P, N], f32, tag="s")
                            psB = s_ps.tile([P, N], f32, tag="s")
                            for kh in range(2):
                                sl = slice(kh * 512, kh * 512 + 512)
                                # two heads packed into PE row groups 0-63 / 64-127
                                nc.tensor.matmul(
                                    psA[:, sl],
                                    lhsT=qTs[
                                        0:64, t, qt * P : (qt + 1) * P
                                    ],
                                    rhs=kTs[0:64, t, sl],
                                    start=True,
                                    stop=False,
                                    tile_position=(0, 0),
                                )
                                nc.tensor.matmul(
                                    psB[:, sl],
                                    lhsT=qTs[
                                        64:128, t, qt * P : (qt + 1) * P
                                    ],
                                    rhs=kTs[64:128, t, sl],
                                    start=True,
                                    stop=False,
                                    tile_position=(64, 0),
                                )
                                # additive bias via identity matmul accumulate
                                nc.tensor.matmul(
                                    psA[:, sl],
                                    lhsT=identb[:],
                                    rhs=biass[:, qt, sl],
                                    start=False,
                                    stop=True,
                                )
                                nc.tensor.matmul(
                                    psB[:, sl],
                                    lhsT=identb[:],
                                    rhs=biass[:, qt, sl],
                                    start=False,
                                    stop=True,
                                )
                            eA = e_pool.tile([P, N], f32, tag="e")
                            eB = e_pool.tile([P, N], f32, tag="e")
                            nc.scalar.activation(
                                eA[:],
                                psA[:],
                                Exp,
                                accum_out=sums[:, qt, hA : hA + 1],
                            )
                            nc.scalar.activation(
                                eB[:],
                                psB[:],
                                Exp,
                                accum_out=sums[:, qt, hB : hB + 1],
                            )
                            nc.vector.reciprocal(
                                recips[:, qt, hA : hA + 2],
                                sums[:, qt, hA : hA + 2],
                            )
                            # normalize on DVE (fp32 SBUF tensor_scalar runs
                            # in 2x mode; GpSimd measured 14.7us/tile here -
                            # ~20x slower - so it gets no work at all)
                            nc.vector.tensor_scalar(
                                stA[:, i, :],
                                eA[:],
                                recips[:, qt, hA : hA + 1],
                                None,
                                mult,
                            )
                            nc.vector.tensor_scalar(
                                stB[:, i, :],
                                eB[:],
                                recips[:, qt, hB : hB + 1],
                                None,
                                mult,
                            )
                        # stream normalized attention to DRAM (1MB per DMA,
                        # bf16->f32 cast done by the SWDGE path)
                        nc.gpsimd.dma_start(
                            out=attn_ap[hA, :, 2 * blk : 2 * blk + 2, :],
                            in_=stA[:],
                        )
                        nc.gpsimd.dma_start(
                            out=attn_ap[hB, :, 2 * blk : 2 * blk + 2, :],
                            in_=stB[:],
                        )
                        # transpose the 2 q-tiles x 2 heads into [k, q]
                        # layout via REGULAR matmuls against the identity
                        # (out = st_tile.T @ I, exact for bf16): unlike
                        # transpose-mode these count as matmul activity for
                        # the HAM clock gate, keeping the PE at 2.4 GHz.
                        # (A DMA-xbar version passed CoreSim but produced
                        # garbage on hardware - known sim/HW divergence.)
                        for kt in range(NT):
                            tp = tp_ps.tile([P, 512], f32, tag="tp")
                            for j, st in enumerate((stA, stA, stB, stB)):
                                i = j % 2
                                nc.tensor.matmul(
                                    tp[:, j * P : (j + 1) * P],
                                    lhsT=st[:, i, kt * P : (kt + 1) * P],
                                    rhs=identb[:],
                                    start=True,
                                    stop=True,
                                    skip_group_check=True,
                                )
                            if kt % 2 == 0:
                                nc.scalar.copy(out=eT[:, kt, :], in_=tp[:])
                            else:
                                nc.vector.tensor_copy(
                                    out=eT[:, kt, :], in_=tp[:]
                                )
                        # attn @ v for this 256-wide q block: head A into
                        # cols 0:256, head B into cols 256:512 of one PSUM
                        # bank (partitions 0-63); accumulate over k-tiles.
                        # One start=True clears the bank's has_written bits;
                        # every later matmul overwrites untouched elements
                        # and accumulates written ones, so head B needs no
                        # start flag of its own.
                        pv = pv_ps.tile([64, 512], f32, tag="pv")
                        for kt in range(NT):
                            nc.tensor.matmul(
                                pv[:, 0:256],
                                lhsT=vs[:, kt, t * P : t * P + 64],
                                rhs=eT[:, kt, 0:256],
                                start=(kt == 0),
                                stop=False,
                                skip_group_check=True,
                            )
                            nc.tensor.matmul(
                                pv[:, 256:512],
                                lhsT=vs[:, kt, t * P + 64 : t * P + 128],
                                rhs=eT[:, kt, 256:512],
                                start=False,
                                stop=(kt == NT - 1),
                                skip_group_check=True,
                            )
                        bsl = slice(blk * 256, blk * 256 + 256)
                        nc.any.tensor_copy(
                            out=ctxT[0:64, t, bsl], in_=pv[:, 0:256]
                        )
                        nc.any.tensor_copy(
                            out=tmpB[:, bsl], in_=pv[:, 256:512]
                        )
                        # head B context lives at partitions 0-63; DMA
                        # shifts it to partitions 64-127 of ctxT (engines
                        # can't cross partitions, DMA can). Per-block so the
                        # last pair's shift overlaps its own attention work.
                        nc.sync.dma_start(
                            out=ctxT[64:128, t, bsl], in_=tmpB[:, bsl]
                        )

                # ------------- Phase 3: output projection -------------
                # Emitted inside the attention pool scope: the proj PSUM
                # tiles reuse the scores pool slots, so the first proj
                # matmul only waits on its slot's last exp instead of a
                # whole-phase PSUM pool transition.
                os_pool = ph2.enter_context(tc.tile_pool(name="os", bufs=2))
                out_ap = out_d.ap().rearrange("(o p) c -> p o c", p=P)
                for m in range(NT):
                    ps = s_ps.tile([P, N], f32, tag="s")
                    for off, nn in ((0, 512), (512, 256)):
                        for k in range(CT):
                            nc.tensor.matmul(
                                ps[:, off : off + nn],
                                lhsT=ctxT[:, k, m * P : (m + 1) * P],
                                rhs=wpsb[:, k, off : off + nn],
                                start=(k == 0),
                                stop=(k == CT - 1),
                            )
                    po = os_pool.tile([P, C], f32, tag="os")
                    nc.any.tensor_copy(out=po[:], in_=ps[:, :C])
                    nc.sync.dma_start(out=out_ap[:, m, :], in_=po[:])


    nc.compile()
    return nc


def _install_axon_ntff_hook():
    """Provide antenv.axon_hooks (absent in this image) so that
    run_bass_kernel_spmd(trace=True) can capture NTFF profiles through
    the axon sidechannel. Mirrors trn_agent_boot.trn_boot."""
    import sys
    import types
    import ctypes
    import contextlib

    if "antenv.axon_hooks" in sys.modules:
        return
    so_path = "/opt/axon/libaxon_pjrt.so"
    lib = ctypes.CDLL(so_path)
    if not hasattr(lib, "axon_start_nrt_profile"):
        return
    lib.axon_start_nrt_profile.argtypes = [
        ctypes.POINTER(ctypes.c_int64),
        ctypes.c_size_t,
    ]
    lib.axon_start_nrt_profile.restype = ctypes.c_int64
    lib.axon_stop_nrt_profile.argtypes = [ctypes.c_char_p]
    lib.axon_stop_nrt_profile.restype = ctypes.c_int64

    @contextlib.contextmanager
    def _hook(output_dir, device_ids):
        import jax

        jax.devices()
        if device_ids:
            ids = (ctypes.c_int64 * len(device_ids))(*device_ids)
            rc = lib.axon_start_nrt_profile(ids, len(device_ids))
        else:
            rc = lib.axon_start_nrt_profile(None, 0)
        if rc != 0:
            raise RuntimeError(f"axon_start_nrt_profile rc={rc}")
        try:
            yield
        finally:
            n = lib.axon_stop_nrt_profile(str(output_dir).encode())
            print(f"[kernel] ntff profile: {n} file(s) -> {output_dir}")

    mod = types.ModuleType("antenv.axon_hooks")
    mod.get_axon_ntff_profile_hook = lambda: _hook
    mod.set_axon_ntff_profile_hook = lambda h: None
    sys.modules["antenv.axon_hooks"] = mod


def kernel(x, Wq, Wk, Wv, Wproj, attn_bias):
    global LAST_EXEC_TIME_NS, LAST_RESULTS
    from concourse.bass_utils import run_bass_kernel_spmd

    if "nc" not in _CACHE:
        _CACHE["nc"] = _build()
    nc = _CACHE["nc"]

    x = np.ascontiguousarray(np.asarray(x, dtype=np.float32))
    bias_bf = np.ascontiguousarray(
        np.asarray(attn_bias, dtype=np.float32).astype(ml_dtypes.bfloat16)
    )
    Wq = np.ascontiguousarray(np.asarray(Wq, dtype=np.float32))
    Wk = np.ascontiguousarray(np.asarray(Wk, dtype=np.float32))
    Wv = np.ascontiguousarray(np.asarray(Wv, dtype=np.float32))
    Wproj = np.ascontiguousarray(np.asarray(Wproj, dtype=np.float32))

    in_maps = [
        {
            "xT": np.ascontiguousarray(x[b].T),
            "Wq": Wq,
            "Wk": Wk,
            "Wv": Wv,
            "Wproj": Wproj,
            "biasb": bias_bf,
        }
        for b in range(B)
    ]

    trace = os.environ.get("KERNEL_PROFILE", "0") == "1"
    res = None
    if trace:
        try:
            _install_axon_ntff_hook()
            tdir = os.environ.get("KERNEL_TRACE_DIR") or None
            res = run_bass_kernel_spmd(
                nc, in_maps, core_ids=list(range(B)), trace=True, tmpdir=tdir
            )
            LAST_EXEC_TIME_NS = res.exec_time_ns
        except Exception as exc:  # trace path can fail; retry without
            import traceback

            traceback.print_exc()
            print(f"[kernel] trace run failed ({exc!r}); rerunning untraced")
            res = None
    if res is None:
        res = run_bass_kernel_spmd(nc, in_maps, core_ids=list(range(B)))
    LAST_RESULTS = res

    out = np.stack([np.asarray(res.results[b]["out"]) for b in range(B)])
    attn = np.stack([np.asarray(res.results[b]["attn"]) for b in range(B)])
    return out, attn
